# revision 12
# baseline (speedup 1.0000x reference)
"""Trainium2 Bass kernel for nn_BilinearInterpolation_60670708023631.

Math: the reference pads the (128,128,32) image into a (128,128,65,32) volume
that is zero everywhere except depth slab z=32, trilinearly samples it at
64*64*65 transformed grid points, and sums over the 65 depth samples per
output pixel.  Because the volume is a single slab, each sample reduces to a
2D 4-corner gather weighted by a z-slab weight zw = fz0*[z0==32]+fz1*[z1==32].
The 4 corners always live in the 2x2 patch at (y0, x0), so we gather one
512-byte patch-table row per sample and fold corner selection into 4 weights.

Sharding: 4096 output pixels split across 8 cores (512 each); the patch table
is replicated.  Each core: compute coords/weights/indices on DVE/ACT from the
raw transformation, indirect-DMA-gather 65*512 patches from HBM, multiply by
weights and reduce on DVE.
"""
import numpy as np

import concourse.bass as bass
import concourse.bacc as bacc
import concourse.mybir as mybir
import concourse.tile as tile
from concourse import bass_utils, library_config

P = 128          # partitions
KD = 65          # depth samples per pixel
NS = 4           # pixel slots per partition (512 pixels / 128)
C = 32           # channels
F = NS * KD      # 260 points per partition
N_CORES = 8
OUT_H = OUT_W = 64
H = W = 128

f32 = mybir.dt.float32
i32 = mybir.dt.int32
i16 = mybir.dt.int16
OP = mybir.AluOpType
AF = mybir.ActivationFunctionType

_CACHE: dict = {}


def _build_program(debug_taps=False):
    nc = bacc.Bacc("TRN2", target_bir_lowering=False, debug=False)

    tab = nc.dram_tensor("tab", (H * W, 4 * C), f32, kind="ExternalInput")
    trep = nc.dram_tensor("trep", (P, 12), f32, kind="ExternalInput")
    xg = nc.dram_tensor("xg", (P, NS), f32, kind="ExternalInput")
    yg = nc.dram_tensor("yg", (P, NS), f32, kind="ExternalInput")
    zgr = nc.dram_tensor("zgr", (P, F), f32, kind="ExternalInput")
    scr = nc.dram_tensor("scr", (P, F), i16)  # DRAM bounce for index rewrap
    out_d = nc.dram_tensor("out", (NS, P, C), f32, kind="ExternalOutput")
    if debug_taps:
        dbg_idx = nc.dram_tensor("dbg_idx", (P, F), i16, kind="ExternalOutput")
        dbg_w = nc.dram_tensor("dbg_w", (P, F * 4), f32, kind="ExternalOutput")
        dbg_g = nc.dram_tensor("dbg_g", (P, KD * 4 * C), f32, kind="ExternalOutput")
        dbg_tmp = nc.dram_tensor("dbg_tmp", (P, KD * 4 * C), f32, kind="ExternalOutput")
        dbg_x = nc.dram_tensor("dbg_x", (P, F), f32, kind="ExternalOutput")
        dbg_xt = nc.dram_tensor("dbg_xt", (P, F), f32, kind="ExternalOutput")

    with tile.TileContext(nc) as tc:
        with (
            tc.tile_pool(name="const", bufs=1) as cp,
            tc.tile_pool(name="work", bufs=1) as wp,
            tc.tile_pool(name="gath", bufs=2) as gp,
            tc.tile_pool(name="tmp", bufs=2) as tp,
            tc.tile_pool(name="outp", bufs=2) as op_,
        ):
            nc.gpsimd.load_library(library_config.mlp)

            # ---- load constants
            t_t = cp.tile([P, 12], f32)
            nc.sync.dma_start(out=t_t[:], in_=trep[:])
            xg_t = cp.tile([P, NS], f32)
            nc.sync.dma_start(out=xg_t[:], in_=xg[:])
            yg_t = cp.tile([P, NS], f32)
            nc.sync.dma_start(out=yg_t[:], in_=yg[:])
            zg_t = cp.tile([P, F], f32)
            nc.sync.dma_start(out=zg_t[:], in_=zgr[:])

            def tcol(j):
                return t_t[:, j:j + 1]

            # ---- stage A: coordinates (X, Y, Z) as [P, F] tiles
            # coord = scale*(T[r,0]*xg + T[r,1]*yg + T[r,3] + 1) + scale*T[r,2]*zg
            coords = {}
            for name, row, scale in (("X", 0, 64.0), ("Y", 1, 64.0), ("Z", 2, 32.5)):
                base = wp.tile([P, NS], f32, tag=f"base{name}")
                u = wp.tile([P, NS], f32, tag="scratch4")
                nc.vector.tensor_scalar(out=base[:], in0=xg_t[:],
                                        scalar1=tcol(4 * row + 0), scalar2=None,
                                        op0=OP.mult)
                nc.vector.tensor_scalar(out=u[:], in0=yg_t[:],
                                        scalar1=tcol(4 * row + 1), scalar2=None,
                                        op0=OP.mult)
                nc.vector.tensor_tensor(out=base[:], in0=base[:], in1=u[:], op=OP.add)
                nc.vector.tensor_scalar(out=base[:], in0=base[:],
                                        scalar1=tcol(4 * row + 3), scalar2=1.0,
                                        op0=OP.add, op1=OP.add)
                nc.vector.tensor_scalar(out=base[:], in0=base[:],
                                        scalar1=float(scale), scalar2=None,
                                        op0=OP.mult)
                czs = wp.tile([P, 1], f32, tag=f"cz{name}")
                nc.vector.tensor_scalar(out=czs[:], in0=tcol(4 * row + 2),
                                        scalar1=float(scale), scalar2=None,
                                        op0=OP.mult)
                co = wp.tile([P, F], f32, tag=f"co{name}")
                nc.vector.tensor_scalar(out=co[:], in0=zg_t[:],
                                        scalar1=czs[:, 0:1], scalar2=None,
                                        op0=OP.mult)
                nc.vector.tensor_tensor(
                    out=co[:].rearrange("p (t k) -> p t k", t=NS),
                    in0=co[:].rearrange("p (t k) -> p t k", t=NS),
                    in1=base[:].broadcast_to([P, NS, KD]),
                    op=OP.add)
                coords[name] = co

            # ---- trunc toward zero: sign(x) * floor(|x|)
            # floor(a) for a>=0: r = rne_int(a); floor = r - (r > a)
            def trunc(x, name):
                a = wp.tile([P, F], f32, tag=f"tr_a{name}")
                nc.scalar.activation(out=a[:], in_=x[:], func=AF.Abs)
                ri = wp.tile([P, F], i32, tag=f"tr_ri{name}")
                nc.vector.tensor_copy(out=ri[:], in_=a[:])
                r = wp.tile([P, F], f32, tag=f"tr_r{name}")
                nc.vector.tensor_copy(out=r[:], in_=ri[:])
                g = wp.tile([P, F], f32, tag=f"tr_g{name}")
                nc.vector.tensor_tensor(out=g[:], in0=r[:], in1=a[:], op=OP.is_gt)
                nc.vector.tensor_tensor(out=r[:], in0=r[:], in1=g[:], op=OP.subtract)
                sg = wp.tile([P, F], f32, tag=f"tr_s{name}")
                nc.scalar.activation(out=sg[:], in_=x[:], func=AF.Sign)
                xt = wp.tile([P, F], f32, tag=f"t{name}")
                nc.vector.tensor_tensor(out=xt[:], in0=r[:], in1=sg[:],
                                        op=OP.mult)
                return xt

            Xt = trunc(coords["X"], "X")
            Yt = trunc(coords["Y"], "Y")
            Zt = trunc(coords["Z"], "Z")

            # ---- clips
            def clip0(xt, hi, name):
                o = wp.tile([P, F], f32, tag=f"c0{name}")
                nc.vector.tensor_scalar(out=o[:], in0=xt[:], scalar1=0.0,
                                        scalar2=float(hi), op0=OP.max, op1=OP.min)
                return o

            def clip1(xt, hi, name):
                o = wp.tile([P, F], f32, tag=f"c1{name}")
                nc.vector.tensor_scalar(out=o[:], in0=xt[:], scalar1=1.0,
                                        scalar2=0.0, op0=OP.add, op1=OP.max)
                nc.vector.tensor_scalar(out=o[:], in0=o[:], scalar1=float(hi),
                                        scalar2=None, op0=OP.min)
                return o

            Xf0 = clip0(Xt, 127, "X"); Xf1 = clip1(Xt, 127, "X")
            Yf0 = clip0(Yt, 127, "Y"); Yf1 = clip1(Yt, 127, "Y")
            Zf0 = clip0(Zt, 64, "Z");  Zf1 = clip1(Zt, 64, "Z")

            # ---- gather indices: idx = Yf0*128 + Xf0 (int16)
            idxf = wp.tile([P, F], f32)
            nc.vector.tensor_scalar(out=idxf[:], in0=Yf0[:], scalar1=128.0,
                                    scalar2=None, op0=OP.mult)
            nc.vector.tensor_tensor(out=idxf[:], in0=idxf[:], in1=Xf0[:], op=OP.add)
            idxi = wp.tile([P, F], i16)
            nc.vector.tensor_copy(out=idxi[:], in_=idxf[:])

            # ---- rewrap indices into dma_gather's 16-partition wrapped layout:
            # wrapped[q + 16r, tk*8 + w] = idxi[16w + q, tk]
            nc.sync.dma_start(out=scr[:], in_=idxi[:])
            wT = wp.tile([P, F * 8], i16)
            for r in range(8):
                nc.sync.dma_start(
                    out=wT[16 * r:16 * r + 16, :].rearrange(
                        "q (w tk) -> q w tk", tk=F),
                    in_=bass.AP(scr, 0, [[F, 16], [16 * F, 8], [1, F]]))
            wrp = wp.tile([P, F * 8], i16)
            nc.vector.tensor_copy(
                out=wrp[:].rearrange("p (tk w) -> p w tk", w=8),
                in_=wT[:].rearrange("p (w tk) -> p w tk", tk=F))

            # ---- weights
            def sub(a, b, tag):
                o = wp.tile([P, F], f32, tag=tag)
                nc.vector.tensor_tensor(out=o[:], in0=a[:], in1=b[:], op=OP.subtract)
                return o

            fx0 = sub(Xf1, coords["X"], "fx0"); fx1 = sub(coords["X"], Xf0, "fx1")
            fy0 = sub(Yf1, coords["Y"], "fy0"); fy1 = sub(coords["Y"], Yf0, "fy1")
            fz0 = sub(Zf1, coords["Z"], "fz0"); fz1 = sub(coords["Z"], Zf0, "fz1")
            dx = sub(Xf1, Xf0, "dx"); dy = sub(Yf1, Yf0, "dy")

            # zw = fz0*[Zf0==32] + fz1*[Zf1==32]
            e0 = wp.tile([P, F], f32, tag="e0")
            nc.vector.tensor_scalar(out=e0[:], in0=Zf0[:], scalar1=32.0,
                                    scalar2=None, op0=OP.is_equal)
            nc.vector.tensor_tensor(out=e0[:], in0=e0[:], in1=fz0[:], op=OP.mult)
            e1 = wp.tile([P, F], f32, tag="e1")
            nc.vector.tensor_scalar(out=e1[:], in0=Zf1[:], scalar1=32.0,
                                    scalar2=None, op0=OP.is_equal)
            nc.vector.tensor_tensor(out=e1[:], in0=e1[:], in1=fz1[:], op=OP.mult)
            zw = wp.tile([P, F], f32, tag="zw")
            nc.vector.tensor_tensor(out=zw[:], in0=e0[:], in1=e1[:], op=OP.add)

            # rf0 = (fx0 + (1-dy)*fx1)*zw ; rf1 = dy*fx1*zw
            # cf0 = fy0 + (1-dx)*fy1     ; cf1 = dx*fy1
            rf1 = wp.tile([P, F], f32, tag="rf1")
            nc.vector.tensor_tensor(out=rf1[:], in0=dy[:], in1=fx1[:], op=OP.mult)
            rf0 = wp.tile([P, F], f32, tag="rf0")
            nc.vector.tensor_tensor(out=rf0[:], in0=fx0[:], in1=fx1[:], op=OP.add)
            nc.vector.tensor_tensor(out=rf0[:], in0=rf0[:], in1=rf1[:], op=OP.subtract)
            nc.vector.tensor_tensor(out=rf0[:], in0=rf0[:], in1=zw[:], op=OP.mult)
            nc.vector.tensor_tensor(out=rf1[:], in0=rf1[:], in1=zw[:], op=OP.mult)

            cf1 = wp.tile([P, F], f32, tag="cf1")
            nc.vector.tensor_tensor(out=cf1[:], in0=dx[:], in1=fy1[:], op=OP.mult)
            cf0 = wp.tile([P, F], f32, tag="cf0")
            nc.vector.tensor_tensor(out=cf0[:], in0=fy0[:], in1=fy1[:], op=OP.add)
            nc.vector.tensor_tensor(out=cf0[:], in0=cf0[:], in1=cf1[:], op=OP.subtract)

            # W slots interleaved [t][k][s]
            wfull = wp.tile([P, F * 4], f32)
            for s, (a, b) in enumerate(((rf0, cf0), (rf0, cf1), (rf1, cf0), (rf1, cf1))):
                nc.vector.tensor_tensor(out=wfull[:, s::4], in0=a[:], in1=b[:],
                                        op=OP.mult)

            if debug_taps:
                nc.sync.dma_start(out=dbg_idx[:], in_=idxi[:])
                nc.sync.dma_start(out=dbg_w[:], in_=wfull[:])
                nc.sync.dma_start(out=dbg_x[:], in_=coords["X"][:])
                nc.sync.dma_start(out=dbg_xt[:], in_=Xt[:])

            # ---- rounds: gather + weighted reduce per pixel-slot
            for t in range(NS):
                g = gp.tile([P, KD * 4 * C], f32, tag="g")
                nc.gpsimd.dma_gather(
                    out_ap=g[:].rearrange("p (k e) -> p k e", e=4 * C),
                    in_ap=tab[:],
                    idxs_ap=wrp[:, t * KD * 8:(t + 1) * KD * 8],
                    num_idxs=KD * P,
                    num_idxs_reg=KD * P,
                    elem_size=4 * C,
                    single_packet=False,
                )
                tmp = tp.tile([P, KD * 4 * C], f32, tag="tmp")
                fr = KD * 4  # 260 (k,s) groups
                nc.vector.tensor_tensor(
                    out=tmp[:].rearrange("p (c f) -> p f c", f=fr),
                    in0=g[:].rearrange("p (f c) -> p f c", c=C),
                    in1=wfull[:, t * fr:(t + 1) * fr].broadcast_to([P, fr, C]),
                    op=OP.mult)
                o = op_.tile([P, C], f32, tag="o")
                nc.vector.tensor_reduce(
                    out=o[:], in_=tmp[:].rearrange("p (c f) -> p c f", f=fr),
                    axis=mybir.AxisListType.X, op=OP.add)
                nc.sync.dma_start(out=out_d[t], in_=o[:])
                if debug_taps and t == 0:
                    nc.sync.dma_start(out=dbg_g[:], in_=g[:])
                    nc.sync.dma_start(out=dbg_tmp[:], in_=tmp[:])

    nc.compile()
    return nc


def _host_prep(image, transformation):
    img = np.ascontiguousarray(np.asarray(image, dtype=np.float32)[0])  # (H, W, C)
    T = np.asarray(transformation, dtype=np.float32).reshape(12)

    xp1 = np.minimum(np.arange(W) + 1, W - 1)
    yp1 = np.minimum(np.arange(H) + 1, H - 1)
    tab = np.concatenate(
        [img, img[:, xp1], img[yp1], img[yp1][:, xp1]], axis=2
    ).reshape(H * W, 4 * C)

    x_lin = np.linspace(-1.0, 1.0, OUT_W, dtype=np.float32)
    y_lin = np.linspace(-1.0, 1.0, OUT_H, dtype=np.float32)
    z_lin = np.linspace(-1.0, 1.0, KD, dtype=np.float32)

    trep = np.tile(T[None, :], (P, 1)).astype(np.float32)
    zgr = np.tile(z_lin, (P, NS)).astype(np.float32)

    in_maps = []
    for c in range(N_CORES):
        pix = c * 512 + np.arange(NS)[None, :] * P + np.arange(P)[:, None]  # (P, NS)
        in_maps.append({
            "tab": tab,
            "trep": trep,
            "xg": np.ascontiguousarray(x_lin[pix % OUT_W]),
            "yg": np.ascontiguousarray(y_lin[pix // OUT_W]),
            "zgr": zgr,
        })
    return in_maps


def _run(in_maps, trace=False):
    nc = _CACHE.get("nc")
    if nc is None:
        nc = _build_program()
        _CACHE["nc"] = nc
    res = bass_utils.run_bass_kernel_spmd(
        nc, in_maps, core_ids=list(range(N_CORES)), trace=trace)
    out_full = np.empty((N_CORES * 512, C), dtype=np.float32)
    for c in range(N_CORES):
        out_full[c * 512:(c + 1) * 512] = res.results[c]["out"].reshape(512, C)
    return out_full.reshape(1, OUT_H, OUT_W, C), res


def kernel(image, transformation):
    in_maps = _host_prep(image, transformation)
    out, _ = _run(in_maps, trace=False)
    return out


# revision 14
# speedup vs baseline: 6.2090x; 6.2090x over previous
"""Trainium2 Bass kernel for nn_BilinearInterpolation_60670708023631.

Math: the reference pads the (128,128,32) image into a (128,128,65,32) volume
that is zero everywhere except depth slab z=32, trilinearly samples it at
64*64*65 transformed grid points, and sums over the 65 depth samples per
output pixel.  Because the volume is a single slab, each sample reduces to a
2D 4-corner gather weighted by a z-slab weight zw = fz0*[z0==32]+fz1*[z1==32].
The 4 corners always live in the 2x2 patch at (y0, x0), so we gather one
512-byte patch-table row per sample and fold corner selection into 4 weights.

zw is nonzero only where the (affine in k) z coordinate crosses [31, 33) —
for a given transformation that is a contiguous window of at most
ceil(2/|dz/dk|)+1 of the 65 depth samples per pixel.  The kernel computes the
per-pixel window start on device and gathers/reduces only KW samples per
pixel; KW is chosen host-side from the transformation's z-slope (falling back
to wider windows or the fully dense variant when the slope is shallow), so
the result is exact for every input.

Sharding: 4096 output pixels split across 8 cores (512 each); the patch table
is replicated.
"""
import numpy as np

import concourse.bass as bass
import concourse.bacc as bacc
import concourse.mybir as mybir
import concourse.tile as tile
from concourse import bass_utils, library_config

P = 128          # partitions
KD = 65          # depth samples per pixel
NS = 4           # pixel slots per partition (512 pixels / 128)
C = 32           # channels
N_CORES = 8
OUT_H = OUT_W = 64
H = W = 128

f32 = mybir.dt.float32
i32 = mybir.dt.int32
i16 = mybir.dt.int16
OP = mybir.AluOpType
AF = mybir.ActivationFunctionType

_CACHE: dict = {}


def _build_program(kw, debug_taps=False):
    """kw = depth-window size per pixel; kw == KD means dense (no windowing)."""
    dense = kw == KD
    F = NS * kw                  # gathered points per partition
    nc = bacc.Bacc("TRN2", target_bir_lowering=False, debug=False)

    tab = nc.dram_tensor("tab", (H * W, 4 * C), f32, kind="ExternalInput")
    trep = nc.dram_tensor("trep", (P, 12), f32, kind="ExternalInput")
    xg = nc.dram_tensor("xg", (P, NS), f32, kind="ExternalInput")
    yg = nc.dram_tensor("yg", (P, NS), f32, kind="ExternalInput")
    jr = nc.dram_tensor("jr", (P, F), f32, kind="ExternalInput")  # j-ramp / z-ramp
    scr = nc.dram_tensor("scr", (P, F), i16)  # DRAM bounce for index rewrap
    out_d = nc.dram_tensor("out", (NS, P, C), f32, kind="ExternalOutput")
    if debug_taps:
        dbg_idx = nc.dram_tensor("dbg_idx", (P, F), i16, kind="ExternalOutput")
        dbg_w = nc.dram_tensor("dbg_w", (P, F * 4), f32, kind="ExternalOutput")
        dbg_kst = nc.dram_tensor("dbg_kst", (P, NS), f32, kind="ExternalOutput")
        dbg_z = nc.dram_tensor("dbg_z", (P, F), f32, kind="ExternalOutput")

    with tile.TileContext(nc) as tc:
        with (
            tc.tile_pool(name="const", bufs=1) as cp,
            tc.tile_pool(name="work", bufs=1) as wp,
            tc.tile_pool(name="gath", bufs=2) as gp,
            tc.tile_pool(name="tmp", bufs=2) as tp,
            tc.tile_pool(name="outp", bufs=2) as op_,
        ):
            nc.gpsimd.load_library(library_config.mlp)

            # ---- load constants
            t_t = cp.tile([P, 12], f32)
            nc.sync.dma_start(out=t_t[:], in_=trep[:])
            xg_t = cp.tile([P, NS], f32)
            nc.sync.dma_start(out=xg_t[:], in_=xg[:])
            yg_t = cp.tile([P, NS], f32)
            nc.sync.dma_start(out=yg_t[:], in_=yg[:])
            jr_t = cp.tile([P, F], f32)
            nc.sync.dma_start(out=jr_t[:], in_=jr[:])

            def tcol(j):
                return t_t[:, j:j + 1]

            # floor(v) for any v: r = rne_int(v); floor = r - (r > v)
            def floor_(x, name, shape):
                ri = wp.tile(shape, i32, tag=f"fl_ri{name}")
                nc.vector.tensor_copy(out=ri[:], in_=x[:])
                r = wp.tile(shape, f32, tag=f"fl_r{name}")
                nc.vector.tensor_copy(out=r[:], in_=ri[:])
                g_ = wp.tile(shape, f32, tag=f"fl_g{name}")
                nc.vector.tensor_tensor(out=g_[:], in0=r[:], in1=x[:], op=OP.is_gt)
                nc.vector.tensor_tensor(out=r[:], in0=r[:], in1=g_[:],
                                        op=OP.subtract)
                return r

            # ---- per-slot affine bases: b_row = scale*(T[r,0]*xg+T[r,1]*yg+T[r,3]+1)
            bases = {}
            cs = {}
            for name, row, scale in (("X", 0, 64.0), ("Y", 1, 64.0), ("Z", 2, 32.5)):
                base = wp.tile([P, NS], f32, tag=f"base{name}")
                u = wp.tile([P, NS], f32, tag=f"scr4{name}")
                nc.vector.tensor_scalar(out=base[:], in0=xg_t[:],
                                        scalar1=tcol(4 * row + 0), scalar2=None,
                                        op0=OP.mult)
                nc.vector.tensor_scalar(out=u[:], in0=yg_t[:],
                                        scalar1=tcol(4 * row + 1), scalar2=None,
                                        op0=OP.mult)
                nc.vector.tensor_tensor(out=base[:], in0=base[:], in1=u[:], op=OP.add)
                nc.vector.tensor_scalar(out=base[:], in0=base[:],
                                        scalar1=tcol(4 * row + 3), scalar2=1.0,
                                        op0=OP.add, op1=OP.add)
                nc.vector.tensor_scalar(out=base[:], in0=base[:],
                                        scalar1=float(scale), scalar2=None,
                                        op0=OP.mult)
                cz = wp.tile([P, 1], f32, tag=f"c{name}")
                nc.vector.tensor_scalar(out=cz[:], in0=tcol(4 * row + 2),
                                        scalar1=float(scale), scalar2=None,
                                        op0=OP.mult)
                bases[name] = base   # [P, NS]: coord at zlin=0 ... coord = cz*zlin + base
                cs[name] = cz        # [P, 1]

            if dense:
                # jr holds zlin replicated: coord = cz*jr + base
                kst = None
            else:
                # ---- window start per pixel slot
                # Z(k) = czk*k + Zb,  czk = cZ/32, Zb = base_Z - cZ
                czk = wp.tile([P, 1], f32)
                nc.vector.tensor_scalar(out=czk[:], in0=cs["Z"][:],
                                        scalar1=1.0 / 32.0, scalar2=None,
                                        op0=OP.mult)
                rcz = wp.tile([P, 1], f32)
                nc.vector.reciprocal(out=rcz[:], in_=czk[:])
                zb = wp.tile([P, NS], f32)
                nc.vector.tensor_scalar(out=zb[:], in0=bases["Z"][:],
                                        scalar1=cs["Z"][:, 0:1], scalar2=None,
                                        op0=OP.subtract)
                # a = (31 - Zb)*rcz ; b = (33 - Zb)*rcz ; klo = min(a, b)
                a = wp.tile([P, NS], f32)
                nc.vector.tensor_scalar(out=a[:], in0=zb[:], scalar1=-1.0,
                                        scalar2=31.0, op0=OP.mult, op1=OP.add)
                nc.vector.tensor_scalar(out=a[:], in0=a[:],
                                        scalar1=rcz[:, 0:1], scalar2=None,
                                        op0=OP.mult)
                b = wp.tile([P, NS], f32)
                nc.vector.tensor_scalar(out=b[:], in0=zb[:], scalar1=-1.0,
                                        scalar2=33.0, op0=OP.mult, op1=OP.add)
                nc.vector.tensor_scalar(out=b[:], in0=b[:],
                                        scalar1=rcz[:, 0:1], scalar2=None,
                                        op0=OP.mult)
                nc.vector.tensor_tensor(out=a[:], in0=a[:], in1=b[:], op=OP.min)
                kf = floor_(a, "k", [P, NS])
                kst = wp.tile([P, NS], f32)
                nc.vector.tensor_scalar(out=kf[:], in0=kf[:], scalar1=1.0,
                                        scalar2=0.0, op0=OP.subtract, op1=OP.max)
                nc.vector.tensor_scalar(out=kst[:], in0=kf[:],
                                        scalar1=float(KD - kw), scalar2=None,
                                        op0=OP.min)
                if debug_taps:
                    nc.sync.dma_start(out=dbg_kst[:], in_=kst[:])

            # ---- coordinates [P, F]
            coords = {}
            for name in ("X", "Y", "Z"):
                co = wp.tile([P, F], f32, tag=f"co{name}")
                if dense:
                    # coord = cz*zlin(k) + base
                    nc.vector.tensor_scalar(out=co[:], in0=jr_t[:],
                                            scalar1=cs[name][:, 0:1], scalar2=None,
                                            op0=OP.mult)
                    nc.vector.tensor_tensor(
                        out=co[:].rearrange("p (t k) -> p t k", t=NS),
                        in0=co[:].rearrange("p (t k) -> p t k", t=NS),
                        in1=bases[name][:].broadcast_to([P, NS, kw]),
                        op=OP.add)
                else:
                    # coord = (c/32)*(kst + j) + (base - c)
                    c32 = wp.tile([P, 1], f32, tag=f"c32{name}")
                    nc.vector.tensor_scalar(out=c32[:], in0=cs[name][:],
                                            scalar1=1.0 / 32.0, scalar2=None,
                                            op0=OP.mult)
                    bp = wp.tile([P, NS], f32, tag=f"bp{name}")
                    nc.vector.tensor_scalar(out=bp[:], in0=kst[:],
                                            scalar1=c32[:, 0:1], scalar2=None,
                                            op0=OP.mult)
                    nc.vector.tensor_tensor(out=bp[:], in0=bp[:],
                                            in1=bases[name][:], op=OP.add)
                    nc.vector.tensor_scalar(out=bp[:], in0=bp[:],
                                            scalar1=cs[name][:, 0:1], scalar2=None,
                                            op0=OP.subtract)
                    nc.vector.tensor_scalar(out=co[:], in0=jr_t[:],
                                            scalar1=c32[:, 0:1], scalar2=None,
                                            op0=OP.mult)
                    nc.vector.tensor_tensor(
                        out=co[:].rearrange("p (t k) -> p t k", t=NS),
                        in0=co[:].rearrange("p (t k) -> p t k", t=NS),
                        in1=bp[:].broadcast_to([P, NS, kw]),
                        op=OP.add)
                coords[name] = co
            if debug_taps:
                nc.sync.dma_start(out=dbg_z[:], in_=coords["Z"][:])

            # ---- trunc toward zero: sign(x) * floor(|x|)
            def trunc(x, name):
                a_ = wp.tile([P, F], f32, tag=f"tr_a{name}")
                nc.scalar.activation(out=a_[:], in_=x[:], func=AF.Abs)
                fl = floor_(a_, f"t{name}", [P, F])
                sg = wp.tile([P, F], f32, tag=f"tr_s{name}")
                nc.scalar.activation(out=sg[:], in_=x[:], func=AF.Sign)
                xt = wp.tile([P, F], f32, tag=f"t{name}")
                nc.vector.tensor_tensor(out=xt[:], in0=fl[:], in1=sg[:],
                                        op=OP.mult)
                return xt

            Xt = trunc(coords["X"], "X")
            Yt = trunc(coords["Y"], "Y")
            Zt = trunc(coords["Z"], "Z")

            # ---- clips
            def clip0(xt, hi, name):
                o = wp.tile([P, F], f32, tag=f"c0{name}")
                nc.vector.tensor_scalar(out=o[:], in0=xt[:], scalar1=0.0,
                                        scalar2=float(hi), op0=OP.max, op1=OP.min)
                return o

            def clip1(xt, hi, name):
                o = wp.tile([P, F], f32, tag=f"c1{name}")
                nc.vector.tensor_scalar(out=o[:], in0=xt[:], scalar1=1.0,
                                        scalar2=0.0, op0=OP.add, op1=OP.max)
                nc.vector.tensor_scalar(out=o[:], in0=o[:], scalar1=float(hi),
                                        scalar2=None, op0=OP.min)
                return o

            Xf0 = clip0(Xt, 127, "X"); Xf1 = clip1(Xt, 127, "X")
            Yf0 = clip0(Yt, 127, "Y"); Yf1 = clip1(Yt, 127, "Y")
            Zf0 = clip0(Zt, 64, "Z");  Zf1 = clip1(Zt, 64, "Z")

            # ---- gather indices: idx = Yf0*128 + Xf0 (int16)
            idxf = wp.tile([P, F], f32)
            nc.vector.tensor_scalar(out=idxf[:], in0=Yf0[:], scalar1=128.0,
                                    scalar2=None, op0=OP.mult)
            nc.vector.tensor_tensor(out=idxf[:], in0=idxf[:], in1=Xf0[:], op=OP.add)
            idxi = wp.tile([P, F], i16)
            nc.vector.tensor_copy(out=idxi[:], in_=idxf[:])
            if debug_taps:
                nc.sync.dma_start(out=dbg_idx[:], in_=idxi[:])

            # ---- rewrap indices into dma_gather's 16-partition wrapped layout:
            # wrapped[q + 16r, f*8 + w] = idxi[16w + q, f]
            nc.sync.dma_start(out=scr[:], in_=idxi[:])
            wT = wp.tile([P, F * 8], i16)
            for r in range(8):
                nc.sync.dma_start(
                    out=wT[16 * r:16 * r + 16, :].rearrange(
                        "q (w f) -> q w f", f=F),
                    in_=bass.AP(scr, 0, [[F, 16], [16 * F, 8], [1, F]]))
            wrp = wp.tile([P, F * 8], i16)
            nc.vector.tensor_copy(
                out=wrp[:].rearrange("p (f w) -> p w f", w=8),
                in_=wT[:].rearrange("p (w f) -> p w f", f=F))

            # ---- weights
            def sub(a_, b_, tag):
                o = wp.tile([P, F], f32, tag=tag)
                nc.vector.tensor_tensor(out=o[:], in0=a_[:], in1=b_[:],
                                        op=OP.subtract)
                return o

            fx0 = sub(Xf1, coords["X"], "fx0"); fx1 = sub(coords["X"], Xf0, "fx1")
            fy0 = sub(Yf1, coords["Y"], "fy0"); fy1 = sub(coords["Y"], Yf0, "fy1")
            fz0 = sub(Zf1, coords["Z"], "fz0"); fz1 = sub(coords["Z"], Zf0, "fz1")
            dx = sub(Xf1, Xf0, "dx"); dy = sub(Yf1, Yf0, "dy")

            # zw = fz0*[Zf0==32] + fz1*[Zf1==32]
            e0 = wp.tile([P, F], f32, tag="e0")
            nc.vector.tensor_scalar(out=e0[:], in0=Zf0[:], scalar1=32.0,
                                    scalar2=None, op0=OP.is_equal)
            nc.vector.tensor_tensor(out=e0[:], in0=e0[:], in1=fz0[:], op=OP.mult)
            e1 = wp.tile([P, F], f32, tag="e1")
            nc.vector.tensor_scalar(out=e1[:], in0=Zf1[:], scalar1=32.0,
                                    scalar2=None, op0=OP.is_equal)
            nc.vector.tensor_tensor(out=e1[:], in0=e1[:], in1=fz1[:], op=OP.mult)
            zw = wp.tile([P, F], f32, tag="zw")
            nc.vector.tensor_tensor(out=zw[:], in0=e0[:], in1=e1[:], op=OP.add)

            # rf0 = (fx0 + (1-dy)*fx1)*zw ; rf1 = dy*fx1*zw
            # cf0 = fy0 + (1-dx)*fy1     ; cf1 = dx*fy1
            rf1 = wp.tile([P, F], f32, tag="rf1")
            nc.vector.tensor_tensor(out=rf1[:], in0=dy[:], in1=fx1[:], op=OP.mult)
            rf0 = wp.tile([P, F], f32, tag="rf0")
            nc.vector.tensor_tensor(out=rf0[:], in0=fx0[:], in1=fx1[:], op=OP.add)
            nc.vector.tensor_tensor(out=rf0[:], in0=rf0[:], in1=rf1[:], op=OP.subtract)
            nc.vector.tensor_tensor(out=rf0[:], in0=rf0[:], in1=zw[:], op=OP.mult)
            nc.vector.tensor_tensor(out=rf1[:], in0=rf1[:], in1=zw[:], op=OP.mult)

            cf1 = wp.tile([P, F], f32, tag="cf1")
            nc.vector.tensor_tensor(out=cf1[:], in0=dx[:], in1=fy1[:], op=OP.mult)
            cf0 = wp.tile([P, F], f32, tag="cf0")
            nc.vector.tensor_tensor(out=cf0[:], in0=fy0[:], in1=fy1[:], op=OP.add)
            nc.vector.tensor_tensor(out=cf0[:], in0=cf0[:], in1=cf1[:], op=OP.subtract)

            # W slots interleaved [f][s]
            wfull = wp.tile([P, F * 4], f32)
            for s, (a_, b_) in enumerate(((rf0, cf0), (rf0, cf1), (rf1, cf0), (rf1, cf1))):
                nc.vector.tensor_tensor(out=wfull[:, s::4], in0=a_[:], in1=b_[:],
                                        op=OP.mult)
            if debug_taps:
                nc.sync.dma_start(out=dbg_w[:], in_=wfull[:])

            # ---- gather + weighted reduce
            # dense: one gather per pixel slot; windowed: one gather for all.
            rounds = ([(t * kw, kw) for t in range(NS)] if dense
                      else [(0, NS * kw)])
            for (f0, nf) in rounds:
                g = gp.tile([P, nf * 4 * C], f32, tag="g")
                nc.gpsimd.dma_gather(
                    out_ap=g[:].rearrange("p (k e) -> p k e", e=4 * C),
                    in_ap=tab[:],
                    idxs_ap=wrp[:, f0 * 8:(f0 + nf) * 8],
                    num_idxs=nf * P,
                    num_idxs_reg=nf * P,
                    elem_size=4 * C,
                    single_packet=False,
                )
                fr = nf * 4
                tmp = tp.tile([P, nf * 4 * C], f32, tag="tmp")
                nc.vector.tensor_tensor(
                    out=tmp[:].rearrange("p (c f) -> p f c", f=fr),
                    in0=g[:].rearrange("p (f c) -> p f c", c=C),
                    in1=wfull[:, f0 * 4:(f0 + nf) * 4].broadcast_to([P, fr, C]),
                    op=OP.mult)
                # per-slot reduce over this round's f-range
                tv = tmp[:].rearrange("p (c f) -> p c f", f=fr)
                for t in range(NS):
                    lo = t * kw * 4 - f0 * 4
                    if lo < 0 or lo >= fr:
                        continue
                    o = op_.tile([P, C], f32, tag="o")
                    nc.vector.tensor_reduce(
                        out=o[:], in_=tv[:, :, lo:lo + kw * 4],
                        axis=mybir.AxisListType.X, op=OP.add)
                    nc.sync.dma_start(out=out_d[t], in_=o[:])

    nc.compile()
    return nc


def _pick_kw(transformation):
    T = np.asarray(transformation, dtype=np.float32).reshape(3, 4)
    czk = abs(float(T[2, 2])) * 65.0 / 64.0   # |dz_voxel/dk|
    if czk == 0.0:
        return KD
    width = 2.0 / czk
    for kw in (8, 16, 32):
        if width <= kw - 3:
            return kw
    return KD


def _host_prep(image, transformation, kw):
    img = np.ascontiguousarray(np.asarray(image, dtype=np.float32)[0])  # (H, W, C)
    T = np.asarray(transformation, dtype=np.float32).reshape(12)

    xp1 = np.minimum(np.arange(W) + 1, W - 1)
    yp1 = np.minimum(np.arange(H) + 1, H - 1)
    tab = np.concatenate(
        [img, img[:, xp1], img[yp1], img[yp1][:, xp1]], axis=2
    ).reshape(H * W, 4 * C)

    x_lin = np.linspace(-1.0, 1.0, OUT_W, dtype=np.float32)
    y_lin = np.linspace(-1.0, 1.0, OUT_H, dtype=np.float32)

    trep = np.tile(T[None, :], (P, 1)).astype(np.float32)
    if kw == KD:
        z_lin = np.linspace(-1.0, 1.0, KD, dtype=np.float32)
        jr = np.tile(z_lin, (P, NS)).astype(np.float32)
    else:
        jr = np.tile(np.arange(kw, dtype=np.float32), (P, NS))

    in_maps = []
    for c in range(N_CORES):
        pix = c * 512 + np.arange(NS)[None, :] * P + np.arange(P)[:, None]  # (P, NS)
        in_maps.append({
            "tab": tab,
            "trep": trep,
            "xg": np.ascontiguousarray(x_lin[pix % OUT_W]),
            "yg": np.ascontiguousarray(y_lin[pix // OUT_W]),
            "jr": jr,
        })
    return in_maps


def _run(in_maps, kw, trace=False):
    nc = _CACHE.get(kw)
    if nc is None:
        nc = _build_program(kw)
        _CACHE[kw] = nc
    res = bass_utils.run_bass_kernel_spmd(
        nc, in_maps, core_ids=list(range(N_CORES)), trace=trace)
    out_full = np.empty((N_CORES * 512, C), dtype=np.float32)
    for c in range(N_CORES):
        out_full[c * 512:(c + 1) * 512] = res.results[c]["out"].reshape(512, C)
    return out_full.reshape(1, OUT_H, OUT_W, C), res


def kernel(image, transformation):
    kw = _pick_kw(transformation)
    in_maps = _host_prep(image, transformation, kw)
    out, _ = _run(in_maps, kw, trace=False)
    return out


# revision 16
# speedup vs baseline: 7.0132x; 1.1295x over previous
"""Trainium2 Bass kernel for nn_BilinearInterpolation_60670708023631.

Math: the reference pads the (128,128,32) image into a (128,128,65,32) volume
that is zero everywhere except depth slab z=32, trilinearly samples it at
64*64*65 transformed grid points, and sums over the 65 depth samples per
output pixel.  Because the volume is a single slab, each sample reduces to a
2D 4-corner gather weighted by a z-slab weight zw = fz0*[z0==32]+fz1*[z1==32].
The 4 corners always live in the 2x2 patch at (y0, x0), so we gather one
512-byte patch-table row per sample and fold corner selection into 4 weights.

zw is nonzero only where the (affine in k) z coordinate crosses [31, 33) —
for a given transformation that is a contiguous window of at most
ceil(2/|dz/dk|)+1 of the 65 depth samples per pixel.  The kernel computes the
per-pixel window start on device and gathers/reduces only KW samples per
pixel; KW is chosen host-side from the transformation's z-slope (falling back
to wider windows or the fully dense variant when the slope is shallow), so
the result is exact for every input.

Sharding: 4096 output pixels split across 8 cores (512 each); the patch table
is replicated.
"""
import numpy as np

import concourse.bass as bass
import concourse.bacc as bacc
import concourse.mybir as mybir
import concourse.tile as tile
from concourse import bass_utils, library_config

P = 128          # partitions
KD = 65          # depth samples per pixel
NS = 4           # pixel slots per partition (512 pixels / 128)
C = 32           # channels
N_CORES = 8
OUT_H = OUT_W = 64
H = W = 128

f32 = mybir.dt.float32
i32 = mybir.dt.int32
i16 = mybir.dt.int16
OP = mybir.AluOpType
AF = mybir.ActivationFunctionType

_CACHE: dict = {}


def _build_program(kw, debug_taps=False):
    """kw = depth-window size per pixel; kw == KD means dense (no windowing)."""
    dense = kw == KD
    F = NS * kw                  # gathered points per partition
    nc = bacc.Bacc("TRN2", target_bir_lowering=False, debug=False)

    tab = nc.dram_tensor("tab", (H * W, 4 * C), f32, kind="ExternalInput")
    trep = nc.dram_tensor("trep", (P, 12), f32, kind="ExternalInput")
    xg = nc.dram_tensor("xg", (P, NS), f32, kind="ExternalInput")
    yg = nc.dram_tensor("yg", (P, NS), f32, kind="ExternalInput")
    jr = nc.dram_tensor("jr", (P, F), f32, kind="ExternalInput")  # j-ramp / z-ramp
    scr = nc.dram_tensor("scr", (P, F), i16)  # DRAM bounce for index rewrap
    out_d = nc.dram_tensor("out", (NS, P, C), f32, kind="ExternalOutput")
    if debug_taps:
        dbg_idx = nc.dram_tensor("dbg_idx", (P, F), i16, kind="ExternalOutput")
        dbg_w = nc.dram_tensor("dbg_w", (P, F * 4), f32, kind="ExternalOutput")
        dbg_kst = nc.dram_tensor("dbg_kst", (P, NS), f32, kind="ExternalOutput")
        dbg_z = nc.dram_tensor("dbg_z", (P, F), f32, kind="ExternalOutput")

    with tile.TileContext(nc) as tc:
        with (
            tc.tile_pool(name="const", bufs=1) as cp,
            tc.tile_pool(name="work", bufs=1) as wp,
            tc.tile_pool(name="gath", bufs=2) as gp,
            tc.tile_pool(name="tmp", bufs=2) as tp,
            tc.tile_pool(name="outp", bufs=2) as op_,
        ):
            nc.gpsimd.load_library(library_config.mlp)

            # ---- load constants
            t_t = cp.tile([P, 12], f32)
            nc.sync.dma_start(out=t_t[:], in_=trep[:])
            xg_t = cp.tile([P, NS], f32)
            nc.sync.dma_start(out=xg_t[:], in_=xg[:])
            yg_t = cp.tile([P, NS], f32)
            nc.sync.dma_start(out=yg_t[:], in_=yg[:])
            jr_t = cp.tile([P, F], f32)
            nc.sync.dma_start(out=jr_t[:], in_=jr[:])

            def tcol(j):
                return t_t[:, j:j + 1]

            # floor(v) for any v: r = rne_int(v); floor = r - (r > v)
            def floor_(x, name, shape):
                ri = wp.tile(shape, i32, tag=f"fl_ri{name}")
                nc.vector.tensor_copy(out=ri[:], in_=x[:])
                r = wp.tile(shape, f32, tag=f"fl_r{name}")
                nc.vector.tensor_copy(out=r[:], in_=ri[:])
                g_ = wp.tile(shape, f32, tag=f"fl_g{name}")
                nc.vector.tensor_tensor(out=g_[:], in0=r[:], in1=x[:], op=OP.is_gt)
                nc.vector.tensor_tensor(out=r[:], in0=r[:], in1=g_[:],
                                        op=OP.subtract)
                return r

            # trunc toward zero on a whole tile: sign(x) * floor(|x|)
            def trunc_(x, name, shape):
                a_ = wp.tile(shape, f32, tag=f"tr_a{name}")
                nc.scalar.activation(out=a_[:], in_=x[:], func=AF.Abs)
                fl = floor_(a_, f"t{name}", shape)
                sg = wp.tile(shape, f32, tag=f"tr_s{name}")
                nc.scalar.activation(out=sg[:], in_=x[:], func=AF.Sign)
                xt = wp.tile(shape, f32, tag=f"t{name}")
                nc.vector.tensor_tensor(out=xt[:], in0=fl[:], in1=sg[:],
                                        op=OP.mult)
                return xt

            # ---- per-slot affine bases: b_row = scale*(T[r,0]*xg+T[r,1]*yg+T[r,3]+1)
            bases = {}
            cs = {}
            for name, row, scale in (("X", 0, 64.0), ("Y", 1, 64.0), ("Z", 2, 32.5)):
                base = wp.tile([P, NS], f32, tag=f"base{name}")
                u = wp.tile([P, NS], f32, tag=f"scr4{name}")
                nc.vector.tensor_scalar(out=base[:], in0=xg_t[:],
                                        scalar1=tcol(4 * row + 0), scalar2=None,
                                        op0=OP.mult)
                nc.vector.tensor_scalar(out=u[:], in0=yg_t[:],
                                        scalar1=tcol(4 * row + 1), scalar2=None,
                                        op0=OP.mult)
                nc.vector.tensor_tensor(out=base[:], in0=base[:], in1=u[:], op=OP.add)
                nc.vector.tensor_scalar(out=base[:], in0=base[:],
                                        scalar1=tcol(4 * row + 3), scalar2=1.0,
                                        op0=OP.add, op1=OP.add)
                nc.vector.tensor_scalar(out=base[:], in0=base[:],
                                        scalar1=float(scale), scalar2=None,
                                        op0=OP.mult)
                cz = wp.tile([P, 1], f32, tag=f"c{name}")
                nc.vector.tensor_scalar(out=cz[:], in0=tcol(4 * row + 2),
                                        scalar1=float(scale), scalar2=None,
                                        op0=OP.mult)
                bases[name] = base   # [P, NS]: coord = cz*zlin + base
                cs[name] = cz        # [P, 1]

            if not dense:
                # ---- window start per pixel slot
                # Z(k) = czk*k + Zb,  czk = cZ/32, Zb = base_Z - cZ
                czk = wp.tile([P, 1], f32)
                nc.vector.tensor_scalar(out=czk[:], in0=cs["Z"][:],
                                        scalar1=1.0 / 32.0, scalar2=None,
                                        op0=OP.mult)
                rcz = wp.tile([P, 1], f32)
                nc.vector.reciprocal(out=rcz[:], in_=czk[:])
                zb = wp.tile([P, NS], f32)
                nc.vector.tensor_scalar(out=zb[:], in0=bases["Z"][:],
                                        scalar1=cs["Z"][:, 0:1], scalar2=None,
                                        op0=OP.subtract)
                # a = (31 - Zb)*rcz ; b = (33 - Zb)*rcz ; klo = min(a, b)
                a = wp.tile([P, NS], f32)
                nc.vector.tensor_scalar(out=a[:], in0=zb[:], scalar1=-1.0,
                                        scalar2=31.0, op0=OP.mult, op1=OP.add)
                nc.vector.tensor_scalar(out=a[:], in0=a[:],
                                        scalar1=rcz[:, 0:1], scalar2=None,
                                        op0=OP.mult)
                b = wp.tile([P, NS], f32)
                nc.vector.tensor_scalar(out=b[:], in0=zb[:], scalar1=-1.0,
                                        scalar2=33.0, op0=OP.mult, op1=OP.add)
                nc.vector.tensor_scalar(out=b[:], in0=b[:],
                                        scalar1=rcz[:, 0:1], scalar2=None,
                                        op0=OP.mult)
                nc.vector.tensor_tensor(out=a[:], in0=a[:], in1=b[:], op=OP.min)
                kf = floor_(a, "k", [P, NS])
                kst = wp.tile([P, NS], f32)
                nc.vector.tensor_scalar(out=kf[:], in0=kf[:], scalar1=1.0,
                                        scalar2=0.0, op0=OP.subtract, op1=OP.max)
                nc.vector.tensor_scalar(out=kst[:], in0=kf[:],
                                        scalar1=float(KD - kw), scalar2=None,
                                        op0=OP.min)
                if debug_taps:
                    nc.sync.dma_start(out=dbg_kst[:], in_=kst[:])

            # ---- coordinates, batched: CO = [X | Y | Z] as [P, 3F]
            CO = wp.tile([P, 3 * F], f32)
            for ci, name in enumerate(("X", "Y", "Z")):
                co = CO[:, ci * F:(ci + 1) * F]
                if dense:
                    # coord = cz*zlin(k) + base
                    nc.vector.tensor_scalar(out=co, in0=jr_t[:],
                                            scalar1=cs[name][:, 0:1], scalar2=None,
                                            op0=OP.mult)
                    nc.vector.tensor_tensor(
                        out=co.rearrange("p (t k) -> p t k", t=NS),
                        in0=co.rearrange("p (t k) -> p t k", t=NS),
                        in1=bases[name][:].broadcast_to([P, NS, kw]),
                        op=OP.add)
                else:
                    # coord = (c/32)*(kst + j) + (base - c)
                    c32 = wp.tile([P, 1], f32, tag=f"c32{name}")
                    nc.vector.tensor_scalar(out=c32[:], in0=cs[name][:],
                                            scalar1=1.0 / 32.0, scalar2=None,
                                            op0=OP.mult)
                    bp = wp.tile([P, NS], f32, tag=f"bp{name}")
                    nc.vector.tensor_scalar(out=bp[:], in0=kst[:],
                                            scalar1=c32[:, 0:1], scalar2=None,
                                            op0=OP.mult)
                    nc.vector.tensor_tensor(out=bp[:], in0=bp[:],
                                            in1=bases[name][:], op=OP.add)
                    nc.vector.tensor_scalar(out=bp[:], in0=bp[:],
                                            scalar1=cs[name][:, 0:1], scalar2=None,
                                            op0=OP.subtract)
                    nc.vector.tensor_scalar(out=co, in0=jr_t[:],
                                            scalar1=c32[:, 0:1], scalar2=None,
                                            op0=OP.mult)
                    nc.vector.tensor_tensor(
                        out=co.rearrange("p (t k) -> p t k", t=NS),
                        in0=co.rearrange("p (t k) -> p t k", t=NS),
                        in1=bp[:].broadcast_to([P, NS, kw]),
                        op=OP.add)
            if debug_taps:
                nc.sync.dma_start(out=dbg_z[:], in_=CO[:, 2 * F:3 * F])

            # ---- trunc + clip0 (batched over XYZ), then gather indices ASAP
            T3 = trunc_(CO, "all", [P, 3 * F])
            CF0 = wp.tile([P, 3 * F], f32)
            nc.vector.tensor_scalar(out=CF0[:], in0=T3[:], scalar1=0.0,
                                    scalar2=127.0, op0=OP.max, op1=OP.min)
            nc.vector.tensor_scalar(out=CF0[:, 2 * F:3 * F],
                                    in0=CF0[:, 2 * F:3 * F], scalar1=64.0,
                                    scalar2=None, op0=OP.min)

            # idx = Yf0*128 + Xf0 (int16)
            idxf = wp.tile([P, F], f32)
            nc.vector.tensor_scalar(out=idxf[:], in0=CF0[:, F:2 * F],
                                    scalar1=128.0, scalar2=None, op0=OP.mult)
            nc.vector.tensor_tensor(out=idxf[:], in0=idxf[:],
                                    in1=CF0[:, 0:F], op=OP.add)
            idxi = wp.tile([P, F], i16)
            nc.vector.tensor_copy(out=idxi[:], in_=idxf[:])
            if debug_taps:
                nc.sync.dma_start(out=dbg_idx[:], in_=idxi[:])

            # ---- rewrap indices into dma_gather's 16-partition wrapped layout:
            # wrapped[q + 16r, f*8 + w] = idxi[16w + q, f]
            nc.sync.dma_start(out=scr[:], in_=idxi[:])
            wT = wp.tile([P, F * 8], i16)
            for r in range(8):
                eng = nc.sync if r % 2 == 0 else nc.scalar
                eng.dma_start(
                    out=wT[16 * r:16 * r + 16, :].rearrange(
                        "q (w f) -> q w f", f=F),
                    in_=bass.AP(scr, 0, [[F, 16], [16 * F, 8], [1, F]]))
            wrp = wp.tile([P, F * 8], i16)
            nc.vector.tensor_copy(
                out=wrp[:].rearrange("p (f w) -> p w f", w=8),
                in_=wT[:].rearrange("p (w f) -> p w f", f=F))

            # ---- weights (overlap the gather descriptor generation)
            CF1 = wp.tile([P, 3 * F], f32)
            nc.vector.tensor_scalar(out=CF1[:], in0=T3[:], scalar1=1.0,
                                    scalar2=0.0, op0=OP.add, op1=OP.max)
            nc.vector.tensor_scalar(out=CF1[:], in0=CF1[:], scalar1=127.0,
                                    scalar2=None, op0=OP.min)
            nc.vector.tensor_scalar(out=CF1[:, 2 * F:3 * F],
                                    in0=CF1[:, 2 * F:3 * F], scalar1=64.0,
                                    scalar2=None, op0=OP.min)

            FB0 = wp.tile([P, 3 * F], f32)   # [fx0 | fy0 | fz0]
            nc.vector.tensor_tensor(out=FB0[:], in0=CF1[:], in1=CO[:],
                                    op=OP.subtract)
            FB1 = wp.tile([P, 3 * F], f32)   # [fx1 | fy1 | fz1]
            nc.vector.tensor_tensor(out=FB1[:], in0=CO[:], in1=CF0[:],
                                    op=OP.subtract)
            DXY = wp.tile([P, 2 * F], f32)   # [dx | dy]
            nc.vector.tensor_tensor(out=DXY[:], in0=CF1[:, 0:2 * F],
                                    in1=CF0[:, 0:2 * F], op=OP.subtract)

            fx0, fx1 = FB0[:, 0:F], FB1[:, 0:F]
            fy0, fy1 = FB0[:, F:2 * F], FB1[:, F:2 * F]
            fz0, fz1 = FB0[:, 2 * F:3 * F], FB1[:, 2 * F:3 * F]
            dx, dy = DXY[:, 0:F], DXY[:, F:2 * F]

            # zw = fz0*[Zf0==32] + fz1*[Zf1==32]
            e0 = wp.tile([P, F], f32, tag="e0")
            nc.vector.tensor_scalar(out=e0[:], in0=CF0[:, 2 * F:3 * F],
                                    scalar1=32.0, scalar2=None, op0=OP.is_equal)
            nc.vector.tensor_tensor(out=e0[:], in0=e0[:], in1=fz0, op=OP.mult)
            e1 = wp.tile([P, F], f32, tag="e1")
            nc.vector.tensor_scalar(out=e1[:], in0=CF1[:, 2 * F:3 * F],
                                    scalar1=32.0, scalar2=None, op0=OP.is_equal)
            nc.vector.tensor_tensor(out=e1[:], in0=e1[:], in1=fz1, op=OP.mult)
            zw = wp.tile([P, F], f32, tag="zw")
            nc.vector.tensor_tensor(out=zw[:], in0=e0[:], in1=e1[:], op=OP.add)

            # rf0 = (fx0 + (1-dy)*fx1)*zw ; rf1 = dy*fx1*zw
            # cf0 = fy0 + (1-dx)*fy1     ; cf1 = dx*fy1
            rf1 = wp.tile([P, F], f32, tag="rf1")
            nc.vector.tensor_tensor(out=rf1[:], in0=dy, in1=fx1, op=OP.mult)
            rf0 = wp.tile([P, F], f32, tag="rf0")
            nc.vector.tensor_tensor(out=rf0[:], in0=fx0, in1=fx1, op=OP.add)
            nc.vector.tensor_tensor(out=rf0[:], in0=rf0[:], in1=rf1[:],
                                    op=OP.subtract)
            nc.vector.tensor_tensor(out=rf0[:], in0=rf0[:], in1=zw[:], op=OP.mult)
            nc.vector.tensor_tensor(out=rf1[:], in0=rf1[:], in1=zw[:], op=OP.mult)

            cf1 = wp.tile([P, F], f32, tag="cf1")
            nc.vector.tensor_tensor(out=cf1[:], in0=dx, in1=fy1, op=OP.mult)
            cf0 = wp.tile([P, F], f32, tag="cf0")
            nc.vector.tensor_tensor(out=cf0[:], in0=fy0, in1=fy1, op=OP.add)
            nc.vector.tensor_tensor(out=cf0[:], in0=cf0[:], in1=cf1[:],
                                    op=OP.subtract)

            # W slots interleaved [f][s]
            wfull = wp.tile([P, F * 4], f32)
            for s, (a_, b_) in enumerate(((rf0, cf0), (rf0, cf1),
                                          (rf1, cf0), (rf1, cf1))):
                nc.vector.tensor_tensor(out=wfull[:, s::4], in0=a_[:], in1=b_[:],
                                        op=OP.mult)
            if debug_taps:
                nc.sync.dma_start(out=dbg_w[:], in_=wfull[:])

            # ---- gather + weighted reduce, chunked so DMA/desc-gen overlaps
            # the multiply/reduce of the previous chunk
            if dense:
                rounds = [(t * kw, kw) for t in range(NS)]
            else:
                rounds = [(0, 2 * kw), (2 * kw, 2 * kw)]
            for (f0, nf) in rounds:
                g = gp.tile([P, nf * 4 * C], f32, tag="g")
                nc.gpsimd.dma_gather(
                    out_ap=g[:].rearrange("p (k e) -> p k e", e=4 * C),
                    in_ap=tab[:],
                    idxs_ap=wrp[:, f0 * 8:(f0 + nf) * 8],
                    num_idxs=nf * P,
                    num_idxs_reg=nf * P,
                    elem_size=4 * C,
                    single_packet=False,
                )
                fr = nf * 4
                tmp = tp.tile([P, nf * 4 * C], f32, tag="tmp")
                nc.vector.tensor_tensor(
                    out=tmp[:].rearrange("p (c f) -> p f c", f=fr),
                    in0=g[:].rearrange("p (f c) -> p f c", c=C),
                    in1=wfull[:, f0 * 4:(f0 + nf) * 4].broadcast_to([P, fr, C]),
                    op=OP.mult)
                if debug_taps and f0 == 0:
                    nc.sync.dma_start(out=dbg_g[:], in_=g[:])
                    nc.sync.dma_start(out=dbg_tmp[:], in_=tmp[:])
                # per-slot reduce over this round's f-range
                tv = tmp[:].rearrange("p (c f) -> p c f", f=fr)
                for t in range(NS):
                    lo = t * kw * 4 - f0 * 4
                    if lo < 0 or lo >= fr:
                        continue
                    o = op_.tile([P, C], f32, tag="o")
                    nc.vector.tensor_reduce(
                        out=o[:], in_=tv[:, :, lo:lo + kw * 4],
                        axis=mybir.AxisListType.X, op=OP.add)
                    nc.sync.dma_start(out=out_d[t], in_=o[:])

    nc.compile()
    return nc


def _pick_kw(transformation):
    T = np.asarray(transformation, dtype=np.float32).reshape(3, 4)
    czk = abs(float(T[2, 2])) * 65.0 / 64.0   # |dz_voxel/dk|
    if czk == 0.0:
        return KD
    width = 2.0 / czk
    for kw in (8, 16, 32):
        if width <= kw - 3:
            return kw
    return KD


def _host_prep(image, transformation, kw):
    img = np.ascontiguousarray(np.asarray(image, dtype=np.float32)[0])  # (H, W, C)
    T = np.asarray(transformation, dtype=np.float32).reshape(12)

    xp1 = np.minimum(np.arange(W) + 1, W - 1)
    yp1 = np.minimum(np.arange(H) + 1, H - 1)
    tab = np.concatenate(
        [img, img[:, xp1], img[yp1], img[yp1][:, xp1]], axis=2
    ).reshape(H * W, 4 * C)

    x_lin = np.linspace(-1.0, 1.0, OUT_W, dtype=np.float32)
    y_lin = np.linspace(-1.0, 1.0, OUT_H, dtype=np.float32)

    trep = np.tile(T[None, :], (P, 1)).astype(np.float32)
    if kw == KD:
        z_lin = np.linspace(-1.0, 1.0, KD, dtype=np.float32)
        jr = np.tile(z_lin, (P, NS)).astype(np.float32)
    else:
        jr = np.tile(np.arange(kw, dtype=np.float32), (P, NS))

    in_maps = []
    for c in range(N_CORES):
        pix = c * 512 + np.arange(NS)[None, :] * P + np.arange(P)[:, None]  # (P, NS)
        in_maps.append({
            "tab": tab,
            "trep": trep,
            "xg": np.ascontiguousarray(x_lin[pix % OUT_W]),
            "yg": np.ascontiguousarray(y_lin[pix // OUT_W]),
            "jr": jr,
        })
    return in_maps


def _run(in_maps, kw, trace=False):
    nc = _CACHE.get(kw)
    if nc is None:
        nc = _build_program(kw)
        _CACHE[kw] = nc
    res = bass_utils.run_bass_kernel_spmd(
        nc, in_maps, core_ids=list(range(N_CORES)), trace=trace)
    out_full = np.empty((N_CORES * 512, C), dtype=np.float32)
    for c in range(N_CORES):
        out_full[c * 512:(c + 1) * 512] = res.results[c]["out"].reshape(512, C)
    return out_full.reshape(1, OUT_H, OUT_W, C), res


def kernel(image, transformation):
    kw = _pick_kw(transformation)
    in_maps = _host_prep(image, transformation, kw)
    out, _ = _run(in_maps, kw, trace=False)
    return out


# revision 17
# speedup vs baseline: 8.4447x; 1.2041x over previous
"""Trainium2 Bass kernel for nn_BilinearInterpolation_60670708023631.

Math: the reference pads the (128,128,32) image into a (128,128,65,32) volume
that is zero everywhere except depth slab z=32, trilinearly samples it at
64*64*65 transformed grid points, and sums over the 65 depth samples per
output pixel.  Because the volume is a single slab, each sample reduces to a
2D 4-corner gather weighted by a z-slab weight zw = fz0*[z0==32]+fz1*[z1==32].
The 4 corners always live in the 2x2 patch at (y0, x0), so we gather one
512-byte patch-table row per sample and fold corner selection into 4 weights.

zw is nonzero only where the (affine in k) z coordinate crosses [31, 33) —
for a given transformation that is a contiguous window of at most
ceil(2/|dz/dk|)+1 of the 65 depth samples per pixel.  The kernel computes the
per-pixel window start on device and gathers/reduces only KW samples per
pixel; KW is chosen host-side from the transformation's z-slope (falling back
to wider windows or the fully dense variant when the slope is shallow), so
the result is exact for every input.

Sharding: 4096 output pixels split across 8 cores (512 each); the patch table
is replicated.
"""
import numpy as np

import concourse.bass as bass
import concourse.bacc as bacc
import concourse.mybir as mybir
import concourse.tile as tile
from concourse import bass_utils, library_config

P = 128          # partitions
KD = 65          # depth samples per pixel
NS = 4           # pixel slots per partition (512 pixels / 128)
C = 32           # channels
N_CORES = 8
OUT_H = OUT_W = 64
H = W = 128

f32 = mybir.dt.float32
i32 = mybir.dt.int32
i16 = mybir.dt.int16
OP = mybir.AluOpType
AF = mybir.ActivationFunctionType

_CACHE: dict = {}


def _build_program(kw, debug_taps=False):
    """kw = depth-window size per pixel; kw == KD means dense (no windowing)."""
    dense = kw == KD
    F = NS * kw                  # gathered points per partition
    nc = bacc.Bacc("TRN2", target_bir_lowering=False, debug=False)

    tab = nc.dram_tensor("tab", (H * W, 4 * C), f32, kind="ExternalInput")
    trep = nc.dram_tensor("trep", (P, 16), f32, kind="ExternalInput")
    xg = nc.dram_tensor("xg", (P, NS), f32, kind="ExternalInput")
    yg = nc.dram_tensor("yg", (P, NS), f32, kind="ExternalInput")
    jr = nc.dram_tensor("jr", (P, F), f32, kind="ExternalInput")  # j-ramp / z-ramp
    scr = nc.dram_tensor("scr", (P, F), i16)  # DRAM bounce for index rewrap
    out_d = nc.dram_tensor("out", (NS, P, C), f32, kind="ExternalOutput")
    if debug_taps:
        dbg_idx = nc.dram_tensor("dbg_idx", (P, F), i16, kind="ExternalOutput")
        dbg_w = nc.dram_tensor("dbg_w", (P, F * 4), f32, kind="ExternalOutput")
        dbg_kst = nc.dram_tensor("dbg_kst", (P, NS), f32, kind="ExternalOutput")
        dbg_z = nc.dram_tensor("dbg_z", (P, F), f32, kind="ExternalOutput")

    with tile.TileContext(nc) as tc:
        with (
            tc.tile_pool(name="const", bufs=1) as cp,
            tc.tile_pool(name="work", bufs=1) as wp,
            tc.tile_pool(name="gath", bufs=2) as gp,
            tc.tile_pool(name="tmp", bufs=2) as tp,
            tc.tile_pool(name="outp", bufs=2) as op_,
        ):
            nc.gpsimd.load_library(library_config.mlp)

            # ---- load constants
            t_t = cp.tile([P, 16], f32)
            nc.sync.dma_start(out=t_t[:], in_=trep[:])
            xg_t = cp.tile([P, NS], f32)
            nc.scalar.dma_start(out=xg_t[:], in_=xg[:])
            yg_t = cp.tile([P, NS], f32)
            nc.sync.dma_start(out=yg_t[:], in_=yg[:])
            jr_t = cp.tile([P, F], f32)
            nc.scalar.dma_start(out=jr_t[:], in_=jr[:])

            def tcol(j):
                return t_t[:, j:j + 1]

            # floor(v) for any v: r = rne_int(v); floor = r - (r > v)
            def floor_(x, name, shape):
                ri = wp.tile(shape, i32, tag=f"fl_ri{name}")
                nc.vector.tensor_copy(out=ri[:], in_=x[:])
                r = wp.tile(shape, f32, tag=f"fl_r{name}")
                nc.vector.tensor_copy(out=r[:], in_=ri[:])
                g_ = wp.tile(shape, f32, tag=f"fl_g{name}")
                nc.vector.tensor_tensor(out=g_[:], in0=r[:], in1=x[:], op=OP.is_gt)
                nc.vector.tensor_tensor(out=r[:], in0=r[:], in1=g_[:],
                                        op=OP.subtract)
                return r

            # trunc toward zero on a whole tile: sign(x) * floor(|x|)
            def trunc_(x, name, shape):
                a_ = wp.tile(shape, f32, tag=f"tr_a{name}")
                nc.scalar.activation(out=a_[:], in_=x[:], func=AF.Abs)
                fl = floor_(a_, f"t{name}", shape)
                sg = wp.tile(shape, f32, tag=f"tr_s{name}")
                nc.scalar.activation(out=sg[:], in_=x[:], func=AF.Sign)
                xt = wp.tile(shape, f32, tag=f"t{name}")
                nc.vector.tensor_tensor(out=xt[:], in0=fl[:], in1=sg[:],
                                        op=OP.mult)
                return xt

            # ceil(v) for any v: r = rne_int(v); ceil = r + (r < v)
            def ceil_(x, name, shape):
                ri = wp.tile(shape, i32, tag=f"cl_ri{name}")
                nc.vector.tensor_copy(out=ri[:], in_=x[:])
                r = wp.tile(shape, f32, tag=f"cl_r{name}")
                nc.vector.tensor_copy(out=r[:], in_=ri[:])
                g_ = wp.tile(shape, f32, tag=f"cl_g{name}")
                nc.vector.tensor_tensor(out=g_[:], in0=r[:], in1=x[:], op=OP.is_lt)
                nc.vector.tensor_tensor(out=r[:], in0=r[:], in1=g_[:], op=OP.add)
                return r

            # ---- per-slot affine bases (prescaled cols from host):
            # pre_r = A*xg + B*yg + D - cfull  (= coord at k=0; windowed)
            # base_r = A*xg + B*yg + D         (= coord at zlin=0; dense)
            pres = {}
            for ci, name in enumerate(("X", "Y", "Z")):
                o = 5 * ci
                base = wp.tile([P, NS], f32, tag=f"base{name}")
                u = wp.tile([P, NS], f32, tag=f"scr4{name}")
                nc.vector.tensor_scalar(out=base[:], in0=xg_t[:],
                                        scalar1=tcol(o + 0), scalar2=None,
                                        op0=OP.mult)
                nc.vector.tensor_scalar(out=u[:], in0=yg_t[:],
                                        scalar1=tcol(o + 1), scalar2=None,
                                        op0=OP.mult)
                nc.vector.tensor_tensor(out=base[:], in0=base[:], in1=u[:],
                                        op=OP.add)
                if dense:
                    nc.vector.tensor_scalar(out=base[:], in0=base[:],
                                            scalar1=tcol(o + 2), scalar2=None,
                                            op0=OP.add)
                else:
                    nc.vector.tensor_scalar(out=base[:], in0=base[:],
                                            scalar1=tcol(o + 2),
                                            scalar2=tcol(o + 4),
                                            op0=OP.add, op1=OP.subtract)
                pres[name] = base

            if not dense:
                # ---- window start per pixel slot: Z(k) = czk*k + Z0
                # klo = min((31-Z0)*rcz, (33-Z0)*rcz); kst = clip(ceil(klo))
                z0 = pres["Z"]
                a = wp.tile([P, NS], f32)
                nc.vector.tensor_scalar(out=a[:], in0=z0[:], scalar1=-1.0,
                                        scalar2=31.0, op0=OP.mult, op1=OP.add)
                nc.vector.tensor_scalar(out=a[:], in0=a[:],
                                        scalar1=tcol(15), scalar2=None,
                                        op0=OP.mult)
                b = wp.tile([P, NS], f32)
                nc.vector.tensor_scalar(out=b[:], in0=z0[:], scalar1=-1.0,
                                        scalar2=33.0, op0=OP.mult, op1=OP.add)
                nc.vector.tensor_scalar(out=b[:], in0=b[:],
                                        scalar1=tcol(15), scalar2=None,
                                        op0=OP.mult)
                nc.vector.tensor_tensor(out=a[:], in0=a[:], in1=b[:], op=OP.min)
                kc = ceil_(a, "k", [P, NS])
                kst = wp.tile([P, NS], f32)
                nc.vector.tensor_scalar(out=kst[:], in0=kc[:], scalar1=0.0,
                                        scalar2=float(KD - kw), op0=OP.max,
                                        op1=OP.min)
                if debug_taps:
                    nc.sync.dma_start(out=dbg_kst[:], in_=kst[:])

            # ---- coordinates, batched: CO = [X | Y | Z] as [P, 3F]
            CO = wp.tile([P, 3 * F], f32)
            for ci, name in enumerate(("X", "Y", "Z")):
                o = 5 * ci
                co = CO[:, ci * F:(ci + 1) * F]
                if dense:
                    # coord = cfull*zlin(k) + base
                    nc.vector.tensor_scalar(out=co, in0=jr_t[:],
                                            scalar1=tcol(o + 4), scalar2=None,
                                            op0=OP.mult)
                    nc.vector.tensor_tensor(
                        out=co.rearrange("p (t k) -> p t k", t=NS),
                        in0=co.rearrange("p (t k) -> p t k", t=NS),
                        in1=pres[name][:].broadcast_to([P, NS, kw]),
                        op=OP.add)
                else:
                    # coord = c32*(kst + j) + pre
                    bp = wp.tile([P, NS], f32, tag=f"bp{name}")
                    nc.vector.tensor_scalar(out=bp[:], in0=kst[:],
                                            scalar1=tcol(o + 3), scalar2=None,
                                            op0=OP.mult)
                    nc.vector.tensor_tensor(out=bp[:], in0=bp[:],
                                            in1=pres[name][:], op=OP.add)
                    nc.vector.tensor_scalar(out=co, in0=jr_t[:],
                                            scalar1=tcol(o + 3), scalar2=None,
                                            op0=OP.mult)
                    nc.vector.tensor_tensor(
                        out=co.rearrange("p (t k) -> p t k", t=NS),
                        in0=co.rearrange("p (t k) -> p t k", t=NS),
                        in1=bp[:].broadcast_to([P, NS, kw]),
                        op=OP.add)
            if debug_taps:
                nc.sync.dma_start(out=dbg_z[:], in_=CO[:, 2 * F:3 * F])

            # ---- trunc + clip0 (batched over XYZ), then gather indices ASAP
            T3 = trunc_(CO, "all", [P, 3 * F])
            CF0 = wp.tile([P, 3 * F], f32)
            nc.vector.tensor_scalar(out=CF0[:], in0=T3[:], scalar1=0.0,
                                    scalar2=127.0, op0=OP.max, op1=OP.min)
            nc.vector.tensor_scalar(out=CF0[:, 2 * F:3 * F],
                                    in0=CF0[:, 2 * F:3 * F], scalar1=64.0,
                                    scalar2=None, op0=OP.min)

            # idx = Yf0*128 + Xf0 (int16)
            idxf = wp.tile([P, F], f32)
            nc.vector.tensor_scalar(out=idxf[:], in0=CF0[:, F:2 * F],
                                    scalar1=128.0, scalar2=None, op0=OP.mult)
            nc.vector.tensor_tensor(out=idxf[:], in0=idxf[:],
                                    in1=CF0[:, 0:F], op=OP.add)
            idxi = wp.tile([P, F], i16)
            nc.vector.tensor_copy(out=idxi[:], in_=idxf[:])
            if debug_taps:
                nc.sync.dma_start(out=dbg_idx[:], in_=idxi[:])

            # ---- rewrap indices into dma_gather's 16-partition wrapped layout:
            # wrapped[q + 16r, f*8 + w] = idxi[16w + q, f]
            nc.sync.dma_start(out=scr[:], in_=idxi[:])
            wT = wp.tile([P, F * 8], i16)
            for r in range(8):
                eng = nc.sync if r % 2 == 0 else nc.scalar
                eng.dma_start(
                    out=wT[16 * r:16 * r + 16, :].rearrange(
                        "q (w f) -> q w f", f=F),
                    in_=bass.AP(scr, 0, [[F, 16], [16 * F, 8], [1, F]]))
            wrp = wp.tile([P, F * 8], i16)
            nc.vector.tensor_copy(
                out=wrp[:].rearrange("p (f w) -> p w f", w=8),
                in_=wT[:].rearrange("p (w f) -> p w f", f=F))

            # ---- weights (overlap the gather descriptor generation)
            CF1 = wp.tile([P, 3 * F], f32)
            nc.vector.tensor_scalar(out=CF1[:], in0=T3[:], scalar1=1.0,
                                    scalar2=0.0, op0=OP.add, op1=OP.max)
            nc.vector.tensor_scalar(out=CF1[:], in0=CF1[:], scalar1=127.0,
                                    scalar2=None, op0=OP.min)
            nc.vector.tensor_scalar(out=CF1[:, 2 * F:3 * F],
                                    in0=CF1[:, 2 * F:3 * F], scalar1=64.0,
                                    scalar2=None, op0=OP.min)

            FB0 = wp.tile([P, 3 * F], f32)   # [fx0 | fy0 | fz0]
            nc.vector.tensor_tensor(out=FB0[:], in0=CF1[:], in1=CO[:],
                                    op=OP.subtract)
            FB1 = wp.tile([P, 3 * F], f32)   # [fx1 | fy1 | fz1]
            nc.vector.tensor_tensor(out=FB1[:], in0=CO[:], in1=CF0[:],
                                    op=OP.subtract)
            DXY = wp.tile([P, 2 * F], f32)   # [dx | dy]
            nc.vector.tensor_tensor(out=DXY[:], in0=CF1[:, 0:2 * F],
                                    in1=CF0[:, 0:2 * F], op=OP.subtract)

            fx0, fx1 = FB0[:, 0:F], FB1[:, 0:F]
            fy0, fy1 = FB0[:, F:2 * F], FB1[:, F:2 * F]
            fz0, fz1 = FB0[:, 2 * F:3 * F], FB1[:, 2 * F:3 * F]
            dx, dy = DXY[:, 0:F], DXY[:, F:2 * F]

            # zw = fz0*[Zf0==32] + fz1*[Zf1==32]
            e0 = wp.tile([P, F], f32, tag="e0")
            nc.vector.tensor_scalar(out=e0[:], in0=CF0[:, 2 * F:3 * F],
                                    scalar1=32.0, scalar2=None, op0=OP.is_equal)
            nc.vector.tensor_tensor(out=e0[:], in0=e0[:], in1=fz0, op=OP.mult)
            e1 = wp.tile([P, F], f32, tag="e1")
            nc.vector.tensor_scalar(out=e1[:], in0=CF1[:, 2 * F:3 * F],
                                    scalar1=32.0, scalar2=None, op0=OP.is_equal)
            nc.vector.tensor_tensor(out=e1[:], in0=e1[:], in1=fz1, op=OP.mult)
            zw = wp.tile([P, F], f32, tag="zw")
            nc.vector.tensor_tensor(out=zw[:], in0=e0[:], in1=e1[:], op=OP.add)

            # rf0 = (fx0 + (1-dy)*fx1)*zw ; rf1 = dy*fx1*zw
            # cf0 = fy0 + (1-dx)*fy1     ; cf1 = dx*fy1
            rf1 = wp.tile([P, F], f32, tag="rf1")
            nc.vector.tensor_tensor(out=rf1[:], in0=dy, in1=fx1, op=OP.mult)
            rf0 = wp.tile([P, F], f32, tag="rf0")
            nc.vector.tensor_tensor(out=rf0[:], in0=fx0, in1=fx1, op=OP.add)
            nc.vector.tensor_tensor(out=rf0[:], in0=rf0[:], in1=rf1[:],
                                    op=OP.subtract)
            nc.vector.tensor_tensor(out=rf0[:], in0=rf0[:], in1=zw[:], op=OP.mult)
            nc.vector.tensor_tensor(out=rf1[:], in0=rf1[:], in1=zw[:], op=OP.mult)

            cf1 = wp.tile([P, F], f32, tag="cf1")
            nc.vector.tensor_tensor(out=cf1[:], in0=dx, in1=fy1, op=OP.mult)
            cf0 = wp.tile([P, F], f32, tag="cf0")
            nc.vector.tensor_tensor(out=cf0[:], in0=fy0, in1=fy1, op=OP.add)
            nc.vector.tensor_tensor(out=cf0[:], in0=cf0[:], in1=cf1[:],
                                    op=OP.subtract)

            # W slots interleaved [f][s]
            wfull = wp.tile([P, F * 4], f32)
            for s, (a_, b_) in enumerate(((rf0, cf0), (rf0, cf1),
                                          (rf1, cf0), (rf1, cf1))):
                nc.vector.tensor_tensor(out=wfull[:, s::4], in0=a_[:], in1=b_[:],
                                        op=OP.mult)
            if debug_taps:
                nc.sync.dma_start(out=dbg_w[:], in_=wfull[:])

            # ---- gather + weighted reduce, chunked so DMA/desc-gen overlaps
            # the multiply/reduce of the previous chunk
            if dense:
                rounds = [(t * kw, kw) for t in range(NS)]
            else:
                rounds = [(0, 2 * kw), (2 * kw, kw), (3 * kw, kw)]
            for (f0, nf) in rounds:
                g = gp.tile([P, nf * 4 * C], f32, tag="g")
                nc.gpsimd.dma_gather(
                    out_ap=g[:].rearrange("p (k e) -> p k e", e=4 * C),
                    in_ap=tab[:],
                    idxs_ap=wrp[:, f0 * 8:(f0 + nf) * 8],
                    num_idxs=nf * P,
                    num_idxs_reg=nf * P,
                    elem_size=4 * C,
                    single_packet=False,
                )
                fr = nf * 4
                tmp = tp.tile([P, nf * 4 * C], f32, tag="tmp")
                nc.vector.tensor_tensor(
                    out=tmp[:].rearrange("p (c f) -> p f c", f=fr),
                    in0=g[:].rearrange("p (f c) -> p f c", c=C),
                    in1=wfull[:, f0 * 4:(f0 + nf) * 4].broadcast_to([P, fr, C]),
                    op=OP.mult)
                if debug_taps and f0 == 0:
                    nc.sync.dma_start(out=dbg_g[:], in_=g[:])
                    nc.sync.dma_start(out=dbg_tmp[:], in_=tmp[:])
                # per-slot reduce over this round's f-range
                tv = tmp[:].rearrange("p (c f) -> p c f", f=fr)
                for t in range(NS):
                    lo = t * kw * 4 - f0 * 4
                    if lo < 0 or lo >= fr:
                        continue
                    o = op_.tile([P, C], f32, tag="o")
                    nc.vector.tensor_reduce(
                        out=o[:], in_=tv[:, :, lo:lo + kw * 4],
                        axis=mybir.AxisListType.X, op=OP.add)
                    nc.sync.dma_start(out=out_d[t], in_=o[:])

    nc.compile()
    return nc


def _pick_kw(transformation):
    T = np.asarray(transformation, dtype=np.float32).reshape(3, 4)
    czk = abs(float(T[2, 2])) * 65.0 / 64.0   # |dz_voxel/dk|
    if czk == 0.0:
        return KD
    width = 2.0 / czk
    for kw in (6, 8, 12, 16, 24, 32, 48):
        if width <= kw - 1.5:
            return kw
    return KD


def _host_prep(image, transformation, kw):
    img = np.ascontiguousarray(np.asarray(image, dtype=np.float32)[0])  # (H, W, C)
    T = np.asarray(transformation, dtype=np.float32).reshape(12)

    xp1 = np.minimum(np.arange(W) + 1, W - 1)
    yp1 = np.minimum(np.arange(H) + 1, H - 1)
    tab = np.concatenate(
        [img, img[:, xp1], img[yp1], img[yp1][:, xp1]], axis=2
    ).reshape(H * W, 4 * C)

    x_lin = np.linspace(-1.0, 1.0, OUT_W, dtype=np.float32)
    y_lin = np.linspace(-1.0, 1.0, OUT_H, dtype=np.float32)

    # prescaled transform columns: per coord r (scale s_r):
    # [A,B,D,c32,cfull] = [s*T[r,0], s*T[r,1], s*(T[r,3]+1), s*T[r,2]/32, s*T[r,2]]
    Tm = T.reshape(3, 4)
    cols = []
    for r, s in ((0, 64.0), (1, 64.0), (2, 32.5)):
        cols += [s * Tm[r, 0], s * Tm[r, 1], s * (Tm[r, 3] + 1.0),
                 s * Tm[r, 2] / 32.0, s * Tm[r, 2]]
    czk = np.float32(Tm[2, 2] * 32.5 / 32.0)
    cols.append(np.float32(1.0) / czk if czk != 0 else np.float32(0.0))
    tvec = np.array(cols, dtype=np.float32)
    trep = np.tile(tvec[None, :], (P, 1)).astype(np.float32)
    if kw == KD:
        z_lin = np.linspace(-1.0, 1.0, KD, dtype=np.float32)
        jr = np.tile(z_lin, (P, NS)).astype(np.float32)
    else:
        jr = np.tile(np.arange(kw, dtype=np.float32), (P, NS))

    in_maps = []
    for c in range(N_CORES):
        pix = c * 512 + np.arange(NS)[None, :] * P + np.arange(P)[:, None]  # (P, NS)
        in_maps.append({
            "tab": tab,
            "trep": trep,
            "xg": np.ascontiguousarray(x_lin[pix % OUT_W]),
            "yg": np.ascontiguousarray(y_lin[pix // OUT_W]),
            "jr": jr,
        })
    return in_maps


def _run(in_maps, kw, trace=False):
    nc = _CACHE.get(kw)
    if nc is None:
        nc = _build_program(kw)
        _CACHE[kw] = nc
    res = bass_utils.run_bass_kernel_spmd(
        nc, in_maps, core_ids=list(range(N_CORES)), trace=trace)
    out_full = np.empty((N_CORES * 512, C), dtype=np.float32)
    for c in range(N_CORES):
        out_full[c * 512:(c + 1) * 512] = res.results[c]["out"].reshape(512, C)
    return out_full.reshape(1, OUT_H, OUT_W, C), res


def kernel(image, transformation):
    kw = _pick_kw(transformation)
    in_maps = _host_prep(image, transformation, kw)
    out, _ = _run(in_maps, kw, trace=False)
    return out


# revision 18
# speedup vs baseline: 8.6693x; 1.0266x over previous
"""Trainium2 Bass kernel for nn_BilinearInterpolation_60670708023631.

Math: the reference pads the (128,128,32) image into a (128,128,65,32) volume
that is zero everywhere except depth slab z=32, trilinearly samples it at
64*64*65 transformed grid points, and sums over the 65 depth samples per
output pixel.  Because the volume is a single slab, each sample reduces to a
2D 4-corner gather weighted by a z-slab weight zw = fz0*[z0==32]+fz1*[z1==32].
The 4 corners always live in the 2x2 patch at (y0, x0), so we gather one
512-byte patch-table row per sample and fold corner selection into 4 weights.

zw is nonzero only where the (affine in k) z coordinate crosses [31, 33) —
for a given transformation that is a contiguous window of at most
ceil(2/|dz/dk|)+1 of the 65 depth samples per pixel.  The kernel computes the
per-pixel window start on device and gathers/reduces only KW samples per
pixel; KW is chosen host-side from the transformation's z-slope (falling back
to wider windows or the fully dense variant when the slope is shallow), so
the result is exact for every input.

Sharding: 4096 output pixels split across 8 cores (512 each); the patch table
is replicated.
"""
import numpy as np

import concourse.bass as bass
import concourse.bacc as bacc
import concourse.mybir as mybir
import concourse.tile as tile
from concourse import bass_utils, library_config

P = 128          # partitions
KD = 65          # depth samples per pixel
NS = 4           # pixel slots per partition (512 pixels / 128)
C = 32           # channels
N_CORES = 8
OUT_H = OUT_W = 64
H = W = 128

f32 = mybir.dt.float32
i32 = mybir.dt.int32
i16 = mybir.dt.int16
OP = mybir.AluOpType
AF = mybir.ActivationFunctionType

_CACHE: dict = {}


def _build_program(kw, debug_taps=False):
    """kw = depth-window size per pixel; kw == KD means dense (no windowing)."""
    dense = kw == KD
    F = NS * kw                  # gathered points per partition
    nc = bacc.Bacc("TRN2", target_bir_lowering=False, debug=False)

    tab = nc.dram_tensor("tab", (H * W, 4 * C), f32, kind="ExternalInput")
    trep = nc.dram_tensor("trep", (P, 16), f32, kind="ExternalInput")
    xg = nc.dram_tensor("xg", (P, NS), f32, kind="ExternalInput")
    yg = nc.dram_tensor("yg", (P, NS), f32, kind="ExternalInput")
    jr = nc.dram_tensor("jr", (P, F), f32, kind="ExternalInput")  # j-ramp / z-ramp
    scr = nc.dram_tensor("scr", (P, F), i16)  # DRAM bounce for index rewrap
    out_d = nc.dram_tensor("out", (NS, P, C), f32, kind="ExternalOutput")
    if debug_taps:
        dbg_idx = nc.dram_tensor("dbg_idx", (P, F), i16, kind="ExternalOutput")
        dbg_w = nc.dram_tensor("dbg_w", (P, F * 4), f32, kind="ExternalOutput")
        dbg_kst = nc.dram_tensor("dbg_kst", (P, NS), f32, kind="ExternalOutput")
        dbg_z = nc.dram_tensor("dbg_z", (P, F), f32, kind="ExternalOutput")

    with tile.TileContext(nc) as tc:
        with (
            tc.tile_pool(name="const", bufs=1) as cp,
            tc.tile_pool(name="work", bufs=1) as wp,
            tc.tile_pool(name="gath", bufs=2) as gp,
            tc.tile_pool(name="tmp", bufs=2) as tp,
            tc.tile_pool(name="outp", bufs=2) as op_,
        ):
            nc.gpsimd.load_library(library_config.mlp)

            # ---- load constants
            t_t = cp.tile([P, 16], f32)
            nc.sync.dma_start(out=t_t[:], in_=trep[:])
            xg_t = cp.tile([P, NS], f32)
            nc.scalar.dma_start(out=xg_t[:], in_=xg[:])
            yg_t = cp.tile([P, NS], f32)
            nc.sync.dma_start(out=yg_t[:], in_=yg[:])
            jr_t = cp.tile([P, F], f32)
            nc.scalar.dma_start(out=jr_t[:], in_=jr[:])

            def tcol(j):
                return t_t[:, j:j + 1]

            # floor(v) for any v: r = rne_int(v); floor = r - (r > v)
            def floor_(x, name, shape):
                ri = wp.tile(shape, i32, tag=f"fl_ri{name}")
                nc.vector.tensor_copy(out=ri[:], in_=x[:])
                r = wp.tile(shape, f32, tag=f"fl_r{name}")
                nc.vector.tensor_copy(out=r[:], in_=ri[:])
                g_ = wp.tile(shape, f32, tag=f"fl_g{name}")
                nc.vector.tensor_tensor(out=g_[:], in0=r[:], in1=x[:], op=OP.is_gt)
                nc.vector.tensor_tensor(out=r[:], in0=r[:], in1=g_[:],
                                        op=OP.subtract)
                return r

            # trunc toward zero on a whole tile: sign(x) * floor(|x|)
            def trunc_(x, name, shape):
                a_ = wp.tile(shape, f32, tag=f"tr_a{name}")
                nc.scalar.activation(out=a_[:], in_=x[:], func=AF.Abs)
                fl = floor_(a_, f"t{name}", shape)
                sg = wp.tile(shape, f32, tag=f"tr_s{name}")
                nc.scalar.activation(out=sg[:], in_=x[:], func=AF.Sign)
                xt = wp.tile(shape, f32, tag=f"t{name}")
                nc.vector.tensor_tensor(out=xt[:], in0=fl[:], in1=sg[:],
                                        op=OP.mult)
                return xt

            # ceil(v) for any v: r = rne_int(v); ceil = r + (r < v)
            def ceil_(x, name, shape):
                ri = wp.tile(shape, i32, tag=f"cl_ri{name}")
                nc.vector.tensor_copy(out=ri[:], in_=x[:])
                r = wp.tile(shape, f32, tag=f"cl_r{name}")
                nc.vector.tensor_copy(out=r[:], in_=ri[:])
                g_ = wp.tile(shape, f32, tag=f"cl_g{name}")
                nc.vector.tensor_tensor(out=g_[:], in0=r[:], in1=x[:], op=OP.is_lt)
                nc.vector.tensor_tensor(out=r[:], in0=r[:], in1=g_[:], op=OP.add)
                return r

            # ---- per-slot affine bases (prescaled cols from host):
            # pre_r = A*xg + B*yg + D - cfull  (= coord at k=0; windowed)
            # base_r = A*xg + B*yg + D         (= coord at zlin=0; dense)
            pres = {}
            for ci, name in enumerate(("X", "Y", "Z")):
                o = 5 * ci
                base = wp.tile([P, NS], f32, tag=f"base{name}")
                u = wp.tile([P, NS], f32, tag=f"scr4{name}")
                nc.vector.tensor_scalar(out=base[:], in0=xg_t[:],
                                        scalar1=tcol(o + 0), scalar2=None,
                                        op0=OP.mult)
                nc.vector.tensor_scalar(out=u[:], in0=yg_t[:],
                                        scalar1=tcol(o + 1), scalar2=None,
                                        op0=OP.mult)
                nc.vector.tensor_tensor(out=base[:], in0=base[:], in1=u[:],
                                        op=OP.add)
                nc.vector.tensor_scalar(out=base[:], in0=base[:],
                                        scalar1=tcol(o + 2), scalar2=None,
                                        op0=OP.add)
                pres[name] = base

            if not dense:
                # ---- window start per pixel slot: Z(k) = czk*k + Z0
                # klo = min((31-Z0)*rcz, (33-Z0)*rcz); kst = clip(ceil(klo))
                # (kst only SELECTS the window; sample values are computed via
                # the same fp path as the dense variant, so +-1 ulp here only
                # shifts which zero-weight samples pad the window)
                z0 = wp.tile([P, NS], f32)
                nc.vector.tensor_scalar(out=z0[:], in0=pres["Z"][:],
                                        scalar1=tcol(14), scalar2=None,
                                        op0=OP.subtract)
                a = wp.tile([P, NS], f32)
                nc.vector.tensor_scalar(out=a[:], in0=z0[:], scalar1=-1.0,
                                        scalar2=31.0, op0=OP.mult, op1=OP.add)
                nc.vector.tensor_scalar(out=a[:], in0=a[:],
                                        scalar1=tcol(15), scalar2=None,
                                        op0=OP.mult)
                b = wp.tile([P, NS], f32)
                nc.vector.tensor_scalar(out=b[:], in0=z0[:], scalar1=-1.0,
                                        scalar2=33.0, op0=OP.mult, op1=OP.add)
                nc.vector.tensor_scalar(out=b[:], in0=b[:],
                                        scalar1=tcol(15), scalar2=None,
                                        op0=OP.mult)
                nc.vector.tensor_tensor(out=a[:], in0=a[:], in1=b[:], op=OP.min)
                kc = ceil_(a, "k", [P, NS])
                kst = wp.tile([P, NS], f32)
                nc.vector.tensor_scalar(out=kst[:], in0=kc[:], scalar1=0.0,
                                        scalar2=float(KD - kw), op0=OP.max,
                                        op1=OP.min)
                if debug_taps:
                    nc.sync.dma_start(out=dbg_kst[:], in_=kst[:])

            # ---- coordinates, batched: CO = [X | Y | Z] as [P, 3F]
            # always evaluated as coord = cfull*zlin(k) + base — bit-identical
            # to the dense variant regardless of the window position
            if dense:
                zl = jr_t
            else:
                # zlin(kst + j) = (kst + j)/32 - 1  (exact in f32)
                u = wp.tile([P, F], f32)
                nc.vector.tensor_tensor(
                    out=u[:].rearrange("p (t k) -> p t k", t=NS),
                    in0=jr_t[:].rearrange("p (t k) -> p t k", t=NS),
                    in1=kst[:].broadcast_to([P, NS, kw]),
                    op=OP.add)
                zl = wp.tile([P, F], f32)
                nc.vector.tensor_scalar(out=zl[:], in0=u[:],
                                        scalar1=1.0 / 32.0, scalar2=-1.0,
                                        op0=OP.mult, op1=OP.add)
            CO = wp.tile([P, 3 * F], f32)
            for ci, name in enumerate(("X", "Y", "Z")):
                o = 5 * ci
                co = CO[:, ci * F:(ci + 1) * F]
                nc.vector.tensor_scalar(out=co, in0=zl[:],
                                        scalar1=tcol(o + 4), scalar2=None,
                                        op0=OP.mult)
                nc.vector.tensor_tensor(
                    out=co.rearrange("p (t k) -> p t k", t=NS),
                    in0=co.rearrange("p (t k) -> p t k", t=NS),
                    in1=pres[name][:].broadcast_to([P, NS, kw]),
                    op=OP.add)
            if debug_taps:
                nc.sync.dma_start(out=dbg_z[:], in_=CO[:, 2 * F:3 * F])

            # ---- trunc + clip0 (batched over XYZ), then gather indices ASAP
            T3 = trunc_(CO, "all", [P, 3 * F])
            CF0 = wp.tile([P, 3 * F], f32)
            nc.vector.tensor_scalar(out=CF0[:], in0=T3[:], scalar1=0.0,
                                    scalar2=127.0, op0=OP.max, op1=OP.min)
            nc.vector.tensor_scalar(out=CF0[:, 2 * F:3 * F],
                                    in0=CF0[:, 2 * F:3 * F], scalar1=64.0,
                                    scalar2=None, op0=OP.min)

            # idx = Yf0*128 + Xf0 (int16)
            idxf = wp.tile([P, F], f32)
            nc.vector.tensor_scalar(out=idxf[:], in0=CF0[:, F:2 * F],
                                    scalar1=128.0, scalar2=None, op0=OP.mult)
            nc.vector.tensor_tensor(out=idxf[:], in0=idxf[:],
                                    in1=CF0[:, 0:F], op=OP.add)
            idxi = wp.tile([P, F], i16)
            nc.vector.tensor_copy(out=idxi[:], in_=idxf[:])
            if debug_taps:
                nc.sync.dma_start(out=dbg_idx[:], in_=idxi[:])

            # ---- rewrap indices into dma_gather's 16-partition wrapped layout:
            # wrapped[q + 16r, f*8 + w] = idxi[16w + q, f]
            nc.sync.dma_start(out=scr[:], in_=idxi[:])
            wT = wp.tile([P, F * 8], i16)
            for r in range(8):
                eng = nc.sync if r % 2 == 0 else nc.scalar
                eng.dma_start(
                    out=wT[16 * r:16 * r + 16, :].rearrange(
                        "q (w f) -> q w f", f=F),
                    in_=bass.AP(scr, 0, [[F, 16], [16 * F, 8], [1, F]]))
            wrp = wp.tile([P, F * 8], i16)
            nc.vector.tensor_copy(
                out=wrp[:].rearrange("p (f w) -> p w f", w=8),
                in_=wT[:].rearrange("p (w f) -> p w f", f=F))

            # ---- weights (overlap the gather descriptor generation)
            CF1 = wp.tile([P, 3 * F], f32)
            nc.vector.tensor_scalar(out=CF1[:], in0=T3[:], scalar1=1.0,
                                    scalar2=0.0, op0=OP.add, op1=OP.max)
            nc.vector.tensor_scalar(out=CF1[:], in0=CF1[:], scalar1=127.0,
                                    scalar2=None, op0=OP.min)
            nc.vector.tensor_scalar(out=CF1[:, 2 * F:3 * F],
                                    in0=CF1[:, 2 * F:3 * F], scalar1=64.0,
                                    scalar2=None, op0=OP.min)

            FB0 = wp.tile([P, 3 * F], f32)   # [fx0 | fy0 | fz0]
            nc.vector.tensor_tensor(out=FB0[:], in0=CF1[:], in1=CO[:],
                                    op=OP.subtract)
            FB1 = wp.tile([P, 3 * F], f32)   # [fx1 | fy1 | fz1]
            nc.vector.tensor_tensor(out=FB1[:], in0=CO[:], in1=CF0[:],
                                    op=OP.subtract)
            DXY = wp.tile([P, 2 * F], f32)   # [dx | dy]
            nc.vector.tensor_tensor(out=DXY[:], in0=CF1[:, 0:2 * F],
                                    in1=CF0[:, 0:2 * F], op=OP.subtract)

            fx0, fx1 = FB0[:, 0:F], FB1[:, 0:F]
            fy0, fy1 = FB0[:, F:2 * F], FB1[:, F:2 * F]
            fz0, fz1 = FB0[:, 2 * F:3 * F], FB1[:, 2 * F:3 * F]
            dx, dy = DXY[:, 0:F], DXY[:, F:2 * F]

            # zw = fz0*[Zf0==32] + fz1*[Zf1==32]
            e0 = wp.tile([P, F], f32, tag="e0")
            nc.vector.tensor_scalar(out=e0[:], in0=CF0[:, 2 * F:3 * F],
                                    scalar1=32.0, scalar2=None, op0=OP.is_equal)
            nc.vector.tensor_tensor(out=e0[:], in0=e0[:], in1=fz0, op=OP.mult)
            e1 = wp.tile([P, F], f32, tag="e1")
            nc.vector.tensor_scalar(out=e1[:], in0=CF1[:, 2 * F:3 * F],
                                    scalar1=32.0, scalar2=None, op0=OP.is_equal)
            nc.vector.tensor_tensor(out=e1[:], in0=e1[:], in1=fz1, op=OP.mult)
            zw = wp.tile([P, F], f32, tag="zw")
            nc.vector.tensor_tensor(out=zw[:], in0=e0[:], in1=e1[:], op=OP.add)

            # rf0 = (fx0 + (1-dy)*fx1)*zw ; rf1 = dy*fx1*zw
            # cf0 = fy0 + (1-dx)*fy1     ; cf1 = dx*fy1
            rf1 = wp.tile([P, F], f32, tag="rf1")
            nc.vector.tensor_tensor(out=rf1[:], in0=dy, in1=fx1, op=OP.mult)
            rf0 = wp.tile([P, F], f32, tag="rf0")
            nc.vector.tensor_tensor(out=rf0[:], in0=fx0, in1=fx1, op=OP.add)
            nc.vector.tensor_tensor(out=rf0[:], in0=rf0[:], in1=rf1[:],
                                    op=OP.subtract)
            nc.vector.tensor_tensor(out=rf0[:], in0=rf0[:], in1=zw[:], op=OP.mult)
            nc.vector.tensor_tensor(out=rf1[:], in0=rf1[:], in1=zw[:], op=OP.mult)

            cf1 = wp.tile([P, F], f32, tag="cf1")
            nc.vector.tensor_tensor(out=cf1[:], in0=dx, in1=fy1, op=OP.mult)
            cf0 = wp.tile([P, F], f32, tag="cf0")
            nc.vector.tensor_tensor(out=cf0[:], in0=fy0, in1=fy1, op=OP.add)
            nc.vector.tensor_tensor(out=cf0[:], in0=cf0[:], in1=cf1[:],
                                    op=OP.subtract)

            # W slots interleaved [f][s]
            wfull = wp.tile([P, F * 4], f32)
            for s, (a_, b_) in enumerate(((rf0, cf0), (rf0, cf1),
                                          (rf1, cf0), (rf1, cf1))):
                nc.vector.tensor_tensor(out=wfull[:, s::4], in0=a_[:], in1=b_[:],
                                        op=OP.mult)
            if debug_taps:
                nc.sync.dma_start(out=dbg_w[:], in_=wfull[:])

            # ---- gather + weighted reduce, chunked so DMA/desc-gen overlaps
            # the multiply/reduce of the previous chunk
            if dense:
                rounds = [(t * kw, kw) for t in range(NS)]
            else:
                rounds = [(0, 2 * kw), (2 * kw, kw), (3 * kw, kw)]
            for (f0, nf) in rounds:
                g = gp.tile([P, nf * 4 * C], f32, tag="g")
                nc.gpsimd.dma_gather(
                    out_ap=g[:].rearrange("p (k e) -> p k e", e=4 * C),
                    in_ap=tab[:],
                    idxs_ap=wrp[:, f0 * 8:(f0 + nf) * 8],
                    num_idxs=nf * P,
                    num_idxs_reg=nf * P,
                    elem_size=4 * C,
                    single_packet=False,
                )
                fr = nf * 4
                tmp = tp.tile([P, nf * 4 * C], f32, tag="tmp")
                nc.vector.tensor_tensor(
                    out=tmp[:].rearrange("p (c f) -> p f c", f=fr),
                    in0=g[:].rearrange("p (f c) -> p f c", c=C),
                    in1=wfull[:, f0 * 4:(f0 + nf) * 4].broadcast_to([P, fr, C]),
                    op=OP.mult)
                if debug_taps and f0 == 0:
                    nc.sync.dma_start(out=dbg_g[:], in_=g[:])
                    nc.sync.dma_start(out=dbg_tmp[:], in_=tmp[:])
                # per-slot reduce over this round's f-range
                tv = tmp[:].rearrange("p (c f) -> p c f", f=fr)
                for t in range(NS):
                    lo = t * kw * 4 - f0 * 4
                    if lo < 0 or lo >= fr:
                        continue
                    o = op_.tile([P, C], f32, tag="o")
                    nc.vector.tensor_reduce(
                        out=o[:], in_=tv[:, :, lo:lo + kw * 4],
                        axis=mybir.AxisListType.X, op=OP.add)
                    nc.sync.dma_start(out=out_d[t], in_=o[:])

    nc.compile()
    return nc


def _pick_kw(transformation):
    T = np.asarray(transformation, dtype=np.float32).reshape(3, 4)
    czk = abs(float(T[2, 2])) * 65.0 / 64.0   # |dz_voxel/dk|
    if czk == 0.0:
        return KD
    width = 2.0 / czk
    for kw in (6, 8, 12, 16, 24, 32, 48):
        if width <= kw - 1.5:
            return kw
    return KD


def _host_prep(image, transformation, kw):
    img = np.ascontiguousarray(np.asarray(image, dtype=np.float32)[0])  # (H, W, C)
    T = np.asarray(transformation, dtype=np.float32).reshape(12)

    xp1 = np.minimum(np.arange(W) + 1, W - 1)
    yp1 = np.minimum(np.arange(H) + 1, H - 1)
    tab = np.concatenate(
        [img, img[:, xp1], img[yp1], img[yp1][:, xp1]], axis=2
    ).reshape(H * W, 4 * C)

    x_lin = np.linspace(-1.0, 1.0, OUT_W, dtype=np.float32)
    y_lin = np.linspace(-1.0, 1.0, OUT_H, dtype=np.float32)

    # prescaled transform columns: per coord r (scale s_r):
    # [A,B,D,c32,cfull] = [s*T[r,0], s*T[r,1], s*(T[r,3]+1), s*T[r,2]/32, s*T[r,2]]
    Tm = T.reshape(3, 4)
    cols = []
    for r, s in ((0, 64.0), (1, 64.0), (2, 32.5)):
        cols += [s * Tm[r, 0], s * Tm[r, 1], s * (Tm[r, 3] + 1.0),
                 s * Tm[r, 2] / 32.0, s * Tm[r, 2]]
    czk = np.float32(Tm[2, 2] * 32.5 / 32.0)
    cols.append(np.float32(1.0) / czk if czk != 0 else np.float32(0.0))
    tvec = np.array(cols, dtype=np.float32)
    trep = np.tile(tvec[None, :], (P, 1)).astype(np.float32)
    if kw == KD:
        z_lin = np.linspace(-1.0, 1.0, KD, dtype=np.float32)
        jr = np.tile(z_lin, (P, NS)).astype(np.float32)
    else:
        jr = np.tile(np.arange(kw, dtype=np.float32), (P, NS))

    in_maps = []
    for c in range(N_CORES):
        pix = c * 512 + np.arange(NS)[None, :] * P + np.arange(P)[:, None]  # (P, NS)
        in_maps.append({
            "tab": tab,
            "trep": trep,
            "xg": np.ascontiguousarray(x_lin[pix % OUT_W]),
            "yg": np.ascontiguousarray(y_lin[pix // OUT_W]),
            "jr": jr,
        })
    return in_maps


def _run(in_maps, kw, trace=False):
    nc = _CACHE.get(kw)
    if nc is None:
        nc = _build_program(kw)
        _CACHE[kw] = nc
    res = bass_utils.run_bass_kernel_spmd(
        nc, in_maps, core_ids=list(range(N_CORES)), trace=trace)
    out_full = np.empty((N_CORES * 512, C), dtype=np.float32)
    for c in range(N_CORES):
        out_full[c * 512:(c + 1) * 512] = res.results[c]["out"].reshape(512, C)
    return out_full.reshape(1, OUT_H, OUT_W, C), res


def kernel(image, transformation):
    kw = _pick_kw(transformation)
    in_maps = _host_prep(image, transformation, kw)
    out, _ = _run(in_maps, kw, trace=False)
    return out
